# revision 24
# baseline (speedup 1.0000x reference)
"""AdaptiveGatingMetaNet on 8 Trainium2 NeuronCores (Bass/Tile SPMD).

Data-parallel over batch (1024 rows/core); weights replicated. Per core:
  - meta-net h in float32r (1 cyc/row PE, ~1.5e-4 rel err — gate-flip safe),
    coeffs in fp32
  - uncertainty via the Gram trick in bf16 (host-verified: zero gate flips):
    unc^2[b,k] = m_b^T diag(w2k) G diag(w2k) m_b with G = W1 W1^T; stage-2
    mask-dot fused into single tensor_tensor_reduce ops on DVE
  - global max via one 512B AllReduce(max)
  - combination: step 0 dense bf16 with the gate applied to the moving
    operand (column scale commutes through the contraction); steps 1..7 on
    the compacted union of active batch columns (CW=448, measured max 389)
  - task matrices (16MB bf16) are streamed per repeat across three DMA
    issuers (scalar/gpsimd/sync) with a 3-deep buffer ring for overlap
"""
import sys
sys.path.insert(0, "/opt/trn_rl_repo")
import numpy as np
from concourse import bass, bacc, tile, mybir

F32 = mybir.dt.float32
F32R = mybir.dt.float32r
BF16 = mybir.dt.bfloat16
I16 = mybir.dt.int16
AX = mybir.AxisListType
ALU = mybir.AluOpType
ACTF = mybir.ActivationFunctionType

D = 1024
H = 256
K = 8
KT_D = D // 128
KT_H = H // 128
N_CORES = 8
CW_DEFAULT = 416


def build(BT=1024, debug_outs=False, phase="full", repeat=1, CW=CW_DEFAULT,
          upto="full", sim=False):
    """Per-core SPMD kernel.

    phase: "full" | "null" (null = passthrough, for dispatch calibration)
    repeat: run the whole computation N times (timing amortization)
    """
    NB = BT // 128
    NS = BT // 512
    nc = bacc.Bacc("TRN2", target_bir_lowering=False, debug=False,
                   num_devices=N_CORES)

    featT_d = nc.dram_tensor("featT", [128, BT, KT_D], F32, kind="ExternalInput")
    w1t_d = nc.dram_tensor("w1t", [128, KT_D, H], F32, kind="ExternalInput")
    w2t_d = nc.dram_tensor("w2t", [128, KT_H, K], F32, kind="ExternalInput")
    g2p_d = nc.dram_tensor("g2p", [128, KT_H, K // 2, 2 * H], BF16,
                           kind="ExternalInput")
    b1t_d = nc.dram_tensor("b1t", [128, KT_H], F32, kind="ExternalInput")
    b2col_d = nc.dram_tensor("b2col", [K, 1], F32, kind="ExternalInput")
    onesel_d = nc.dram_tensor("onesel", [128, K, 8], BF16, kind="ExternalInput")
    selmat_d = nc.dram_tensor("selmat", [K, D], BF16, kind="ExternalInput")
    ident_d = nc.dram_tensor("ident", [128, 128], F32, kind="ExternalInput")
    identb_d = nc.dram_tensor("identb", [128, 128], BF16, kind="ExternalInput")
    scal_d = nc.dram_tensor("scal", [1, 4], F32, kind="ExternalInput")
    mats_d = nc.dram_tensor("mats", [K, 128, KT_D, D], BF16, kind="ExternalInput")
    iota16_d = nc.dram_tensor("iota16", [16, BT // 16], F32, kind="ExternalInput")
    positer_d = nc.dram_tensor("positer", [16, CW // 16], F32, kind="ExternalInput")
    outc_d = nc.dram_tensor("outc", [128, CW, KT_D], F32, kind="ExternalOutput")
    idxout_d = nc.dram_tensor("idxout", [16, CW // 16], F32, kind="ExternalOutput")
    if debug_outs:
        dbg_gated_d = nc.dram_tensor("dbg_gated", [K, BT], F32, kind="ExternalOutput")
        dbg_u2_d = nc.dram_tensor("dbg_u2", [K, BT], F32, kind="ExternalOutput")

    cc_in = nc.dram_tensor("cc_in", [128], F32)
    cc_out = nc.dram_tensor("cc_out", [128], F32, addr_space="Shared")

    with tile.TileContext(nc) as tc:
        with (
            tc.tile_pool(name="persist", bufs=1) as pp,
            tc.tile_pool(name="work", bufs=2) as wp,
            tc.tile_pool(name="rmats", bufs=2) as mp,
            tc.tile_pool(name="psP", bufs=1, space="PSUM") as psP,
        ):
            # ----- constants (once) -----
            # weights go on the vector/scalar DGE rings so the sync ring
            # starts streaming featT (the h critical path) immediately
            w1t = pp.tile([128, KT_D, H], F32, tag="w1t")
            nc.gpsimd.dma_start(w1t[:], w1t_d[:])
            w2t = pp.tile([128, KT_H, K], F32, tag="w2t")
            nc.gpsimd.dma_start(w2t[:], w2t_d[:])
            b1t = pp.tile([128, KT_H], F32, tag="b1t")
            nc.gpsimd.dma_start(b1t[:], b1t_d[:])
            sel = pp.tile([K, D], BF16, tag="sel")
            nc.scalar.dma_start(sel[:], selmat_d[:])
            ident = pp.tile([128, 128], F32, tag="ident")
            nc.scalar.dma_start(ident[:], ident_d[:])
            identb = pp.tile([128, 128], BF16, tag="identb")
            nc.scalar.dma_start(identb[:], identb_d[:])
            scal = pp.tile([1, 4], F32, tag="scal")
            nc.gpsimd.dma_start(scal[:], scal_d[:])
            w1tr = pp.tile([128, KT_D, H], F32R, tag="w1tr")
            nc.vector.tensor_copy(w1tr[:], w1t[:])
            b2col = pp.tile([K, 1], F32, tag="b2col")
            nc.gpsimd.dma_start(b2col[:], b2col_d[:])
            onesel = pp.tile([128, K, 8], BF16, tag="onesel")
            nc.gpsimd.dma_start(onesel[:], onesel_d[:])
            # G2 pairs (diag(w2k) W1W1^T diag(w2k)) are weight-only: computed
            # on the host, streamed in as one 2MB bf16 input (scalar ring, so
            # the sync ring starts on featT immediately)
            g2all = pp.tile([128, KT_H, K // 2, 2 * H], BF16, tag="g2all")
            nc.scalar.dma_start(g2all[:], g2p_d[:])

            for rep in range(repeat):
                # ----- x load in [128, b, m] layout (m fastest, so the
                # compact gather is ONE d=8 ap_gather); two 2MB DMAs so h's
                # first half starts early
                xTall = pp.tile([128, BT, KT_D], F32, tag="xTall",
                                name=f"xT_{rep}")
                nc.sync.dma_start(xTall[:, 0:BT // 2, :], featT_d[:, 0:BT // 2, :])
                nc.sync.dma_start(xTall[:, BT // 2:, :], featT_d[:, BT // 2:, :])

                if phase == "null":
                    break

                # mats prefetch: 2-deep ring, all on the sync HWDGE ring so
                # no compute engine's instruction FIFO blocks behind them
                matj = []
                for j in range(K):
                    t = mp.tile([128, KT_D, D], BF16, tag="matj",
                                name=f"matj{j}_{rep}")
                    nc.sync.dma_start(t[:], mats_d[j])
                    matj.append(t)

                # ----- h = relu(W1 @ x^T + b1) in f32r; maskT = (h > 0) -----
                relu = pp.tile([128, KT_H, BT], F32, tag="relu", name=f"relu_{rep}")
                maskT = pp.tile([128, KT_H, BT], BF16, tag="maskT",
                                name=f"maskT_{rep}")
                for ns in range(NS):
                    nsl = slice(ns * 512, (ns + 1) * 512)
                    hacc = [psP.tile([128, 512], F32, tag=("B1", "B2")[ms],
                                     bufs=(2, 1)[ms], name=f"h{ms}_{ns}_{rep}")
                            for ms in range(KT_H)]
                    for kt in range(KT_D):
                        xr = wp.tile([128, 512], F32R, tag="xr",
                                     name=f"xr{kt}_{ns}_{rep}")
                        nc.scalar.copy(xr[:], xTall[:, nsl, kt])
                        for ms in range(KT_H):
                            nc.tensor.matmul(
                                hacc[ms][:], w1tr[:, kt, ms * 128:(ms + 1) * 128],
                                xr[:], start=(kt == 0), stop=(kt == KT_D - 1))
                    for ms in range(KT_H):
                        nc.scalar.activation(relu[:, ms, nsl], hacc[ms][:],
                                             ACTF.Relu, bias=b1t[:, ms:ms + 1],
                                             scale=1.0)
                        nc.vector.tensor_scalar(maskT[:, ms, nsl],
                                                relu[:, ms, nsl],
                                                0.0, None, ALU.is_gt)

                # rcur = plain bf16 round of xT (gating-independent, so the
                # j=0 matmuls run while the collective + gating complete)
                rcur = wp.tile([128, KT_D, BT], BF16, tag="rmoving", bufs=1,
                               name=f"r0_{rep}")
                for kt in range(KT_D):
                    for ns in range(NS):
                        nsl = slice(ns * 512, (ns + 1) * 512)
                        nc.vector.tensor_copy(rcur[:, kt, nsl],
                                              xTall[:, nsl, kt])

                # ----- coeffsT = W2 @ relu + b2 (fp32, T layout [K, BT]) -----
                coeffsT = pp.tile([K, BT], F32, tag="coeffsT",
                                  name=f"coeffsT_{rep}")
                for ns in range(NS):
                    nsl = slice(ns * 512, (ns + 1) * 512)
                    cps = psP.tile([K, 512], F32, tag="small", bufs=2,
                                   name=f"cps{ns}_{rep}")
                    for kt in range(KT_H):
                        nc.tensor.matmul(cps[:], w2t[:, kt, :], relu[:, kt, nsl],
                                         start=(kt == 0), stop=(kt == KT_H - 1))
                    nc.scalar.activation(coeffsT[:, nsl], cps[:], ACTF.Identity,
                                         bias=b2col[:], scale=1.0)

                # ----- u2T via transposed stage-1 + PE selector-ones reduce:
                # xkT[h',b] = sum_h G2k[h,h'] mask[h,b]; prod = xkT*mask;
                # u2T[k,b] = sum_h' prod  (accumulated into one [K,512] bank
                # through e_k selector matmuls)
                u2T = pp.tile([K, BT], F32, tag="u2T", name=f"u2T_{rep}")
                for ns in range(NS):
                    nsl = slice(ns * 512, (ns + 1) * 512)
                    u2ps = psP.tile([K, 512], F32, tag="small", bufs=2,
                                    name=f"u2ps{ns}_{rep}")
                    nmm = 0
                    for k in range(K):
                        pair, half = divmod(k, 2)
                        hs = half * H
                        prods = []
                        for ms in range(KT_H):
                            xkt = psP.tile([128, 512], F32,
                                           tag=("B1", "B2")[k % 2],
                                           bufs=(2, 1)[k % 2],
                                           name=f"xkt{k}_{ms}_{ns}_{rep}")
                            for kt in range(KT_H):
                                nc.tensor.matmul(
                                    xkt[:],
                                    g2all[:, kt, pair,
                                          hs + ms * 128:hs + (ms + 1) * 128],
                                    maskT[:, kt, nsl],
                                    start=(kt == 0), stop=(kt == KT_H - 1))
                            prod = wp.tile([128, 512], BF16, tag="prod", bufs=2,
                                           name=f"prod{k}_{ms}_{ns}_{rep}")
                            nc.vector.tensor_tensor(prod[:], xkt[:],
                                                    maskT[:, ms, nsl], ALU.mult)
                            prods.append(prod)
                        for ms in range(KT_H):
                            nc.tensor.matmul(u2ps[:], onesel[:, k, :],
                                             prods[ms][:],
                                             start=(nmm == 0),
                                             stop=(nmm == 2 * K - 1))
                            nmm += 1
                    nc.vector.tensor_copy(u2T[:, nsl], u2ps[:])

                # ----- AllReduce max (high priority: issue the collective
                # the moment u2T is done so peer skew starts draining) -----
                with tc.high_priority():
                    lmax = wp.tile([128, 1], F32, tag="lmax", name=f"lmax_{rep}")
                    nc.vector.memset(lmax[:], 0.0)
                    nc.vector.tensor_reduce(lmax[0:K, :], u2T[:], AX.X, ALU.max)
                    nc.gpsimd.dma_start(cc_in[:], lmax[:])
                    if not sim:
                        nc.gpsimd.collective_compute(
                            "AllReduce", ALU.max,
                            replica_groups=[list(range(N_CORES))],
                            ins=[cc_in[:]], outs=[cc_out[:]])
                    gmax_col = wp.tile([1, 128], F32, tag="gmax_col",
                                       name=f"gmax_{rep}")
                    nc.gpsimd.dma_start(gmax_col[:], cc_in[:] if sim else cc_out[:])
                m2 = wp.tile([1, 1], F32, tag="m2", name=f"m2_{rep}")
                nc.vector.tensor_reduce(m2[:], gmax_col[:], AX.X, ALU.max)
                sq = wp.tile([1, 1], F32, tag="sqm2", name=f"sq_{rep}")
                nc.scalar.activation(sq[:], m2[:], ACTF.Sqrt)
                rs = wp.tile([1, 1], F32, tag="rsq", name=f"rs_{rep}")
                nc.vector.reciprocal(rs[:], sq[:])
                s11 = wp.tile([1, 1], F32, tag="s11", name=f"s11_{rep}")
                nc.vector.tensor_tensor(s11[:], rs[:], scal[:, 1:2], ALU.mult)
                sbc = wp.tile([128, 1], F32, tag="sbc", name=f"sbc_{rep}")
                nc.gpsimd.partition_broadcast(sbc[:], s11[:])
                basebc = wp.tile([128, 1], F32, tag="basebc", name=f"basebc_{rep}")
                nc.gpsimd.partition_broadcast(basebc[:], scal[:, 0:1])

                # ----- thresholds + gating (row-wise in T layout) -----
                sqrT = wp.tile([K, BT], F32, tag="t8", name=f"sqrT_{rep}")
                nc.scalar.activation(sqrT[:], u2T[:], ACTF.Sqrt)
                thT = wp.tile([K, BT], F32, tag="t8", name=f"thT_{rep}")
                nc.scalar.activation(thT[:], sqrT[:], ACTF.Identity,
                                     bias=basebc[0:K, :], scale=sbc[0:K, :])
                absT = pp.tile([K, BT], F32, tag="u2T", name=f"absT_{rep}")
                nc.scalar.activation(absT[:], coeffsT[:], ACTF.Abs)
                keepT = wp.tile([K, BT], F32, tag="keepT", name=f"keepT_{rep}")
                nc.vector.tensor_tensor(keepT[:], absT[:], thT[:], ALU.is_ge)
                gatedT16 = pp.tile([16, BT], F32, tag="gatedT16",
                                   name=f"gatedT16_{rep}")
                nc.vector.memset(gatedT16[:], 0.0)
                nc.vector.tensor_tensor(gatedT16[0:K, :], coeffsT[:], keepT[:],
                                        ALU.mult)
                if debug_outs and rep == repeat - 1:
                    nc.sync.dma_start(dbg_gated_d[:], gatedT16[0:K, :])
                    nc.sync.dma_start(dbg_u2_d[:], u2T[:])
                # gate row 0 broadcast to all partitions (this concourse
                # rejects step-0 partition APs, so materialize it in bf16)
                g0row = wp.tile([1, BT], BF16, tag="g0row", name=f"g0row_{rep}")
                nc.vector.tensor_copy(g0row[:], gatedT16[0:1, :])
                gb0 = pp.tile([128, BT], BF16, tag="gb0", name=f"gb0_{rep}")
                nc.gpsimd.partition_broadcast(gb0[:], g0row[:])

                if upto == "gating":
                    continue
                # ----- combination loop -----
                # j=0 runs DENSE with the gate folded into the bf16 moving
                # operand; j=1..7 run on the compacted active columns (CW).
                WF = CW // 16

                # j=0 dense pass: xT += gb0 * (matj0^T @ rcur)
                for m in range(KT_D):
                    for ns in range(NS):
                        nsl = slice(ns * 512, (ns + 1) * 512)
                        ps = psP.tile([128, 512], F32, tag="A", bufs=3,
                                      name=f"lps0_{m}_{ns}_{rep}")
                        for kt in range(KT_D):
                            nc.tensor.matmul(
                                ps[:], matj[0][:, kt, m * 128:(m + 1) * 128],
                                rcur[:, kt, nsl],
                                start=(kt == 0), stop=(kt == KT_D - 1))
                        tmp = wp.tile([128, 512], BF16, tag="tmp0", bufs=2,
                                      name=f"tmp{m}_{ns}_{rep}")
                        nc.vector.tensor_tensor(
                            tmp[:], ps[:], gb0[:, nsl], ALU.mult)
                        nc.vector.tensor_tensor(xTall[:, nsl, m], xTall[:, nsl, m],
                                                tmp[:], ALU.add)

                # unionK [128, NB] via PE transposes of keepT chunks.
                # wait_until pushes these behind the j=0 dense matmuls in the
                # PE stream: they stall on the collective, and an in-order PE
                # would otherwise idle through it (measured 61us).
                unionK = pp.tile([128, NB], F32, tag="unionK", name=f"unionK_{rep}")
                with tc.tile_wait_until(rep * 1.0 + 0.05):
                    for bt in range(NB):
                        tps = psP.tile([128, K], F32, tag="small", bufs=2,
                                       name=f"tpsu{bt}_{rep}")
                        nc.tensor.transpose(tps[:], keepT[:, bt * 128:(bt + 1) * 128],
                                            ident[0:K, 0:K])
                        nc.vector.tensor_reduce(unionK[:, bt:bt + 1], tps[:],
                                                AX.X, ALU.max)

                # --- index machinery (overlaps the j=0 matmuls above) ---
                iota16 = pp.tile([16, BT // 16], F32, tag="iota16", name=f"iota16_{rep}")
                nc.scalar.dma_start(iota16[:], iota16_d[:])
                positer = pp.tile([16, WF], F32, tag="positer", name=f"positer_{rep}")
                nc.scalar.dma_start(positer[:], positer_d[:])
                u16 = wp.tile([16, NB, 8], F32, tag="u16", name=f"u16_{rep}")
                for g in range(8):
                    nc.scalar.dma_start(u16[:, :, g:g + 1],
                                        unionK[g * 16:(g + 1) * 16, :])
                cand = wp.tile([16, BT // 16], F32, tag="cand", name=f"cand_{rep}")
                # u16 layout: [p16, g, bt] maps to b = bt*128 + g*16 + p16;
                # iota wrapped layout needs b = f*16 + p16 with f = bt*8 + g.
                # Reorder free dims via an AP view: (g, bt) -> (bt, g)
                u16flat = u16[:].rearrange("p b g -> p (b g)")
                nc.vector.tensor_tensor(cand[:], iota16[:], u16flat, ALU.mult)
                um1 = wp.tile([16, BT // 16], F32, tag="um1", name=f"um1_{rep}")
                nc.vector.tensor_scalar(um1[:], u16flat, 1.0, None, ALU.subtract)
                nc.vector.tensor_tensor(cand[:], cand[:], um1[:], ALU.add)
                idxf = wp.tile([16, WF], F32, tag="idxf", name=f"idxf_{rep}")
                cnt = wp.tile([1, 1], mybir.dt.uint32, tag="cnt", name=f"cnt_{rep}")
                nc.gpsimd.sparse_gather(idxf[:], cand[:], num_found=cnt[:])
                # mask junk tail: idx = idx if pos < cnt else -1
                cntf = wp.tile([1, 1], F32, tag="cntf", name=f"cntf_{rep}")
                nc.vector.tensor_copy(cntf[:], cnt[:])
                cntbc = wp.tile([16, 1], F32, tag="cntbc", name=f"cntbc_{rep}")
                nc.gpsimd.partition_broadcast(cntbc[:], cntf[:])
                posok = wp.tile([16, WF], F32, tag="posok", name=f"posok_{rep}")
                nc.vector.tensor_scalar(posok[:], positer[:], cntbc[:], None, ALU.is_lt)
                nc.vector.tensor_tensor(idxf[:], idxf[:], posok[:], ALU.mult)
                pm1 = wp.tile([16, WF], F32, tag="pm1", name=f"pm1_{rep}")
                nc.vector.tensor_scalar(pm1[:], posok[:], 1.0, None, ALU.subtract)
                nc.vector.tensor_tensor(idxf[:], idxf[:], pm1[:], ALU.add)
                nc.scalar.dma_start(idxout_d[:], idxf[:])
                idxf0 = wp.tile([16, WF], F32, tag="idxf0", name=f"idxf0_{rep}")
                nc.vector.tensor_scalar(idxf0[:], idxf[:], 0.0, None, ALU.max)
                idx16 = wp.tile([16, WF], I16, tag="idx16", name=f"idx16_{rep}")
                nc.vector.tensor_copy(idx16[:], idxf0[:])
                idxrep = wp.tile([128, WF], I16, tag="idxrep", name=f"idxrep_{rep}")
                for g in range(8):
                    nc.scalar.dma_start(idxrep[g * 16:(g + 1) * 16, :], idx16[:])

                # gatedTc: compact gated rows
                gatedTc = pp.tile([16, CW], F32, tag="gatedTc", name=f"gatedTc_{rep}")
                nc.gpsimd.ap_gather(gatedTc[:], gatedT16[:], idx16[:], channels=16,
                                    num_elems=BT, d=1, num_idxs=CW)
                # rows 1..7 broadcast to all 128 partitions: bf16 the rows,
                # stage each at partition 0 of gflat (SBUF->SBUF DMA), then
                # gpsimd-broadcast per row so step j's chunk is ready without
                # waiting for all.
                gatedTcb = wp.tile([K, CW], BF16, tag="gatedTcb",
                                   name=f"gatedTcb_{rep}")
                nc.vector.tensor_copy(gatedTcb[:], gatedTc[0:K, :])
                gflat = pp.tile([1, (K - 1) * CW], BF16, tag="gflat",
                                name=f"gflat_{rep}")
                for j in range(1, K):
                    nc.scalar.dma_start(gflat[0:1, (j - 1) * CW:j * CW],
                                        gatedTcb[j:j + 1, :])
                gball = pp.tile([128, (K - 1) * CW], BF16, tag="gball",
                                name=f"gball_{rep}")
                for j in range(1, K):
                    nc.gpsimd.partition_broadcast(
                        gball[:, (j - 1) * CW:j * CW],
                        gflat[0:1, (j - 1) * CW:j * CW])

                # gather compacted x (after j=0 updates) into SBUF: ONE d=8
                # gather (4KB/descriptor) instead of 8 column gathers
                # (512B/descriptor, measured 12.7us EACH)
                xgall = pp.tile([128, CW, KT_D], F32, tag="xgall",
                                name=f"xgall_{rep}")
                nc.gpsimd.ap_gather(xgall[:], xTall[:], idxrep[:], channels=128,
                                    num_elems=BT, d=KT_D, num_idxs=CW)
                xg = [xgall[:, :, m] for m in range(KT_D)]

                # compact loop j=1..7: xc accumulates IN PSUM (one bank per
                # m-tile, reusing every pool tag); the PE accumulates each
                # step's update in place, so the only per-(j,m) DVE work is
                # producing the next step's scaled moving operand rc.
                # The gathered xg is folded in by DVE after step 1's matmuls
                # have set has_written (DVE writes alone would be overwritten).
                if upto == "full":
                    xc_tags = [("B1", 2), ("B1", 2), ("B2", 1), ("small", 2),
                               ("small", 2), ("A", 3), ("A", 3), ("A", 3)]
                    xc = [psP.tile([128, CW], F32, tag=tg, bufs=bf,
                                   name=f"xcp{m}_{rep}")
                          for m, (tg, bf) in enumerate(xc_tags)]
                    # rc ping-pongs between the dead relu slot and one work slot
                    def rc_tile(i, nm):
                        if i % 2 == 0:
                            return pp.tile([128, KT_D, CW], BF16, tag="relu",
                                           name=nm)
                        return wp.tile([128, KT_D, CW], BF16, tag="rc", bufs=1,
                                       name=nm)
                    rc = rc_tile(0, f"rc1_{rep}")
                    for m in range(KT_D):
                        nc.vector.tensor_tensor(
                            rc[:, m, :], xg[m], gball[:, 0:CW], ALU.mult)
                    for j in range(1, K):
                        rcn = None
                        if j + 1 < K:
                            rcn = rc_tile(j, f"rc{j+1}_{rep}")
                        for m in range(KT_D):
                            for kt in range(KT_D):
                                nc.tensor.matmul(
                                    xc[m][:], matj[j][:, kt, m * 128:(m + 1) * 128],
                                    rc[:, kt, :],
                                    start=(j == 1 and kt == 0),
                                    stop=(kt == KT_D - 1))
                            if j == 1:
                                nc.vector.tensor_tensor(xc[m][:], xc[m][:],
                                                        xg[m], ALU.add)
                            if rcn is not None:
                                nc.vector.tensor_tensor(
                                    rcn[:, m, :], xc[m][:],
                                    gball[:, j * CW:(j + 1) * CW], ALU.mult)
                        rc = rcn
                    # stage PSUM results back to SBUF for the output DMA
                    for m in range(KT_D):
                        nc.vector.tensor_copy(xg[m], xc[m][:])

            if phase != "null" and upto != "gating":
                nc.sync.dma_start(outc_d[:], xgall[:])

    nc.finalize()
    return nc


def make_inputs(features_shard_T, W1, b1, W2, b2, task_mats, base_threshold, beta,
                CW=CW_DEFAULT):
    import ml_dtypes
    BT = features_shard_T.shape[1]
    featT = np.ascontiguousarray(
        features_shard_T.reshape(KT_D, 128, BT).transpose(1, 2, 0))
    w1t = np.ascontiguousarray(W1.T.reshape(KT_D, 128, H).transpose(1, 0, 2))
    w2t = np.ascontiguousarray(W2.T.reshape(KT_H, 128, K).transpose(1, 0, 2))
    b1t = np.ascontiguousarray(b1.reshape(KT_H, 128).T)
    sel = np.zeros((K, D), np.float32)
    for j in range(K):
        sel[j, j * 128:(j + 1) * 128] = 1.0
    base_sp = np.log1p(np.exp(np.float32(base_threshold[0]))).astype(np.float32)
    rbeta = np.maximum(np.float32(beta[0]), 0).astype(np.float32)
    scal = np.array([[base_sp, base_sp * rbeta, 0, 0]], np.float32)
    mats = np.ascontiguousarray(
        task_mats.reshape(K, KT_D, 128, D).transpose(0, 2, 1, 3)).astype(ml_dtypes.bfloat16)
    iota16 = (np.arange(BT).reshape(BT // 16, 16).T).astype(np.float32)
    positer = (np.arange(CW).reshape(CW // 16, 16).T).astype(np.float32)
    # host-side weight-only precompute: G2 pairs in the device layout
    # g2p[p, kt, pair, half*H + h'] = W2[k,h] * G[h,h'] * W2[k,h']
    # with h = kt*128 + p, k = 2*pair + half
    G = (W1.astype(np.float32) @ W1.T.astype(np.float32))
    g2p = np.empty((128, KT_H, K // 2, 2 * H), np.float32)
    for pair in range(K // 2):
        for half in range(2):
            k = 2 * pair + half
            G2k = W2[k][:, None] * G * W2[k][None, :]
            for kt in range(KT_H):
                g2p[:, kt, pair, half * H:(half + 1) * H] = \
                    G2k[kt * 128:(kt + 1) * 128, :]
    return {
        "iota16": np.ascontiguousarray(iota16),
        "positer": np.ascontiguousarray(positer),
        "featT": featT.astype(np.float32),
        "w1t": w1t.astype(np.float32),
        "w2t": w2t.astype(np.float32),
        "g2p": g2p.astype(ml_dtypes.bfloat16),
        "b1t": b1t.astype(np.float32),
        "b2col": b2.reshape(K, 1).astype(np.float32),
        "onesel": np.broadcast_to(np.eye(K, dtype=np.float32)[None, :, :],
                                  (128, K, K)).astype(ml_dtypes.bfloat16),
        "selmat": sel.astype(ml_dtypes.bfloat16),
        "ident": np.eye(128, dtype=np.float32),
        "identb": np.eye(128, dtype=np.float32).astype(ml_dtypes.bfloat16),
        "scal": scal,
        "mats": mats,
    }




_CACHE = {}


def _host_reference(features, W1, b1, W2, b2, task_mats, base_threshold, beta):
    """Pure-numpy fallback (only for pathological gating densities)."""
    f = features.astype(np.float64)
    h = f @ W1.T.astype(np.float64) + b1.astype(np.float64)
    relu_h = np.maximum(h, 0.0)
    coeffs = relu_h @ W2.T.astype(np.float64) + b2.astype(np.float64)
    mask = (h > 0).astype(np.float64)
    G = W1.astype(np.float64) @ W1.T.astype(np.float64)
    u2 = np.zeros((f.shape[0], K))
    for k in range(K):
        V = mask * W2[k].astype(np.float64)[None, :]
        u2[:, k] = np.sum((V @ G) * V, axis=1)
    unc = np.sqrt(u2)
    mx = unc.max()
    if mx > 0:
        unc = unc / mx
    base = np.log1p(np.exp(np.float64(base_threshold.reshape(-1)[0])))
    th = base * (1.0 + max(float(beta.reshape(-1)[0]), 0.0) * unc)
    gated = np.where(np.abs(coeffs) < th, 0.0, coeffs)
    x = f.copy()
    for j in range(K):
        x = x + gated[:, j:j + 1] * (x @ task_mats[j].astype(np.float64))
    return x.astype(np.float32)


def _union_counts(features, W1, b1, W2, b2, base_threshold, beta):
    """Approximate per-core union-active counts (sizing check only)."""
    f = features.astype(np.float32)
    h = f @ W1.T + b1
    coeffs = np.maximum(h, 0) @ W2.T + b2
    mask = (h > 0).astype(np.float32)
    G = (W1 @ W1.T).astype(np.float32)
    u2 = np.zeros((f.shape[0], K), np.float32)
    for k in range(K):
        V = mask * W2[k][None, :]
        u2[:, k] = np.sum((V @ G) * V, axis=1)
    unc = np.sqrt(u2)
    mx = unc.max()
    if mx > 0:
        unc = unc / mx
    base = np.log1p(np.exp(np.float32(base_threshold.reshape(-1)[0])))
    th = base * (1 + max(float(beta.reshape(-1)[0]), 0.0) * unc)
    active = (np.abs(coeffs) >= th).any(axis=1)
    BT = f.shape[0] // N_CORES
    return [int(active[c * BT:(c + 1) * BT].sum()) for c in range(N_CORES)]


def kernel(features, W1, b1, W2, b2, task_mats, proj_W, base_threshold, beta,
           **_unused):
    from concourse.bass_utils import run_bass_kernel_spmd

    features = np.asarray(features, dtype=np.float32)
    W1 = np.asarray(W1, np.float32)
    b1 = np.asarray(b1, np.float32)
    W2 = np.asarray(W2, np.float32)
    b2 = np.asarray(b2, np.float32)
    task_mats = np.asarray(task_mats, np.float32)
    base_threshold = np.asarray(base_threshold, np.float32)
    beta = np.asarray(beta, np.float32)
    B = features.shape[0]
    BT = B // N_CORES
    CW = CW_DEFAULT

    # sizing check: the compact loop holds up to CW active columns per core.
    # device/host gate decisions can differ by a couple of borderline samples,
    # so require a safety margin; otherwise fall back to exact host compute.
    counts = _union_counts(features, W1, b1, W2, b2, base_threshold, beta)
    x = None
    if max(counts) <= CW - 24 and B % N_CORES == 0:
        try:
            key = ("nc", BT, CW)
            if key not in _CACHE:
                _CACHE[key] = build(BT=BT, CW=CW)
            nc = _CACHE[key]
            in_maps = []
            for c in range(N_CORES):
                shard_T = np.ascontiguousarray(features[c * BT:(c + 1) * BT].T)
                in_maps.append(make_inputs(shard_T, W1, b1, W2, b2, task_mats,
                                           base_threshold, beta, CW=CW))
            res = run_bass_kernel_spmd(nc, in_maps, core_ids=list(range(N_CORES)))
            outs = []
            for c in range(N_CORES):
                r = res.results[c]
                idx1d = r["idxout"].T.reshape(-1)      # unwrap i = f*16 + p
                valid = idx1d >= 0
                idxs = idx1d[valid].astype(np.int64)
                xcT = r["outc"].transpose(2, 0, 1).reshape(D, CW)
                xfull = features[c * BT:(c + 1) * BT].copy()
                xfull[idxs, :] = xcT[:, valid].T
                outs.append(xfull)
            x = np.concatenate(outs, axis=0)
        except Exception as e:
            import traceback
            print(f"kernel: device path failed ({e!r}); falling back to host",
                  file=sys.stderr)
            traceback.print_exc()
            x = None
    if x is None:
        x = _host_reference(features, W1, b1, W2, b2, task_mats,
                            base_threshold, beta)

    proj_W = np.asarray(proj_W, np.float32)
    if not (proj_W.shape == (D, D) and
            np.array_equal(proj_W, np.eye(D, dtype=proj_W.dtype))):
        x = (x @ proj_W.T).astype(np.float32)
    return np.ascontiguousarray(x.astype(np.float32))



# revision 32
# speedup vs baseline: 1.2123x; 1.2123x over previous
"""AdaptiveGatingMetaNet on 8 Trainium2 NeuronCores (Bass/Tile SPMD).

Data-parallel over batch (1024 rows/core); weights replicated. Per core:
  - meta-net h in float32r (1 cyc/row PE, ~1.5e-4 rel err — gate-flip safe),
    coeffs in fp32
  - uncertainty via the Gram trick in bf16 (host-verified: zero gate flips):
    unc^2[b,k] = m_b^T diag(w2k) G diag(w2k) m_b with G = W1 W1^T
  - global max via one 512B AllReduce(max)
  - combination: ALL 8 steps run on the compacted union of active batch
    columns (CW=416, measured max 389); x is gathered once from the original
    features in a d=8-blocked layout (one 4KB-descriptor ap_gather), and the
    running x accumulates in SBUF f32 (PSUM stays free for the next rep's
    meta-net, enabling cross-rep software pipelining via wait_until tiers)
  - task matrices (16MB bf16) stream on a dedicated sync DGE ring
"""
import sys
sys.path.insert(0, "/opt/trn_rl_repo")
import numpy as np
from concourse import bass, bacc, tile, mybir

F32 = mybir.dt.float32
F32R = mybir.dt.float32r
BF16 = mybir.dt.bfloat16
I16 = mybir.dt.int16
AX = mybir.AxisListType
ALU = mybir.AluOpType
ACTF = mybir.ActivationFunctionType

D = 1024
H = 256
K = 8
KT_D = D // 128
KT_H = H // 128
N_CORES = 8
CW_DEFAULT = 416


def build(BT=1024, debug_outs=False, phase="full", repeat=1, CW=CW_DEFAULT,
          upto="full", sim=False):
    """Per-core SPMD kernel.

    phase: "full" | "null" (null = passthrough, for dispatch calibration)
    repeat: run the whole computation N times (timing amortization)
    """
    NB = BT // 128
    NS = BT // 512
    CWG = 512
    WF = CWG // 16
    WFC = CW // 16
    nc = bacc.Bacc("TRN2", target_bir_lowering=False, debug=False,
                   num_devices=N_CORES)

    featT_d = nc.dram_tensor("featT", [128, BT, KT_D], F32, kind="ExternalInput")
    featB_d = nc.dram_tensor("featB", [BT, D], BF16, kind="ExternalInput")
    w1t_d = nc.dram_tensor("w1t", [128, KT_D, H], F32, kind="ExternalInput")
    w2t_d = nc.dram_tensor("w2t", [128, KT_H, K], F32, kind="ExternalInput")
    g2p_d = nc.dram_tensor("g2p", [128, KT_H, K // 2, 2 * H], BF16,
                           kind="ExternalInput")
    b1t_d = nc.dram_tensor("b1t", [128, KT_H], F32, kind="ExternalInput")
    b2col_d = nc.dram_tensor("b2col", [K, 1], F32, kind="ExternalInput")
    onesel_d = nc.dram_tensor("onesel", [128, K, 8], BF16, kind="ExternalInput")
    ident_d = nc.dram_tensor("ident", [128, 128], F32, kind="ExternalInput")
    scal_d = nc.dram_tensor("scal", [1, 4], F32, kind="ExternalInput")
    mats_d = nc.dram_tensor("mats", [K, 128, KT_D, D], BF16, kind="ExternalInput")
    iota16_d = nc.dram_tensor("iota16", [16, BT // 16], F32, kind="ExternalInput")
    positer_d = nc.dram_tensor("positer", [16, WF], F32, kind="ExternalInput")
    outc_d = nc.dram_tensor("outc", [128, KT_D, CW], F32, kind="ExternalOutput")
    idxout_d = nc.dram_tensor("idxout", [16, WF], F32, kind="ExternalOutput")
    if debug_outs:
        dbg_gated_d = nc.dram_tensor("dbg_gated", [K, BT], F32, kind="ExternalOutput")
        dbg_u2_d = nc.dram_tensor("dbg_u2", [K, BT], F32, kind="ExternalOutput")

    cc_in = nc.dram_tensor("cc_in", [128], F32)
    cc_out = nc.dram_tensor("cc_out", [128], F32, addr_space="Shared")

    # wait_until tiers (logical ms): phase1 of rep r at r*P1; the gating-
    # dependent tail of rep r sorts after phase1 of reps r+1 AND r+2 so the
    # in-order PE fills the collective+machinery latency with the next reps'
    # meta-net matmuls (software pipelining, depth 2).
    P1 = 0.2
    t_p1 = lambda r: r * P1
    t_mach = lambda r: r * P1 + 0.35
    t_comp = lambda r: r * P1 + 0.5

    with tile.TileContext(nc) as tc:
        with (
            tc.tile_pool(name="persist", bufs=1) as pp,
            tc.tile_pool(name="work", bufs=2) as wp,
            tc.tile_pool(name="rmats", bufs=2) as mp,
            tc.tile_pool(name="psP", bufs=1, space="PSUM") as psP,
        ):
            # ----- constants (once): gpsimd/scalar rings; the sync ring is
            # reserved for the 16MB/rep mats stream -----
            w1t = pp.tile([128, KT_D, H], F32, tag="w1t")
            nc.gpsimd.dma_start(w1t[:], w1t_d[:])
            w2t = pp.tile([128, KT_H, K], F32, tag="w2t")
            nc.gpsimd.dma_start(w2t[:], w2t_d[:])
            b1t = pp.tile([128, KT_H], F32, tag="b1t")
            nc.gpsimd.dma_start(b1t[:], b1t_d[:])
            ident = pp.tile([128, 128], F32, tag="ident")
            nc.gpsimd.dma_start(ident[:], ident_d[:])
            scal = pp.tile([1, 4], F32, tag="scal")
            nc.gpsimd.dma_start(scal[:], scal_d[:])
            b2col = pp.tile([K, 1], F32, tag="b2col")
            nc.gpsimd.dma_start(b2col[:], b2col_d[:])
            onesel = pp.tile([128, K, 8], BF16, tag="onesel")
            nc.gpsimd.dma_start(onesel[:], onesel_d[:])
            iota16 = pp.tile([16, BT // 16], F32, tag="iota16")
            nc.gpsimd.dma_start(iota16[:], iota16_d[:])
            positer = pp.tile([16, WF], F32, tag="positer")
            nc.gpsimd.dma_start(positer[:], positer_d[:])
            w1tr = pp.tile([128, KT_D, H], F32R, tag="w1tr")
            nc.vector.tensor_copy(w1tr[:], w1t[:])
            # G2 pairs (diag(w2k) W1W1^T diag(w2k)) are weight-only: computed
            # on the host, streamed in as one 2MB bf16 input
            g2all = pp.tile([128, KT_H, K // 2, 2 * H], BF16, tag="g2all")
            nc.scalar.dma_start(g2all[:], g2p_d[:])

            for rep in range(repeat):
                with tc.tile_wait_until(t_p1(rep)):
                    # ----- x load in [128, b, m] layout (m fastest, so the
                    # compact gather is ONE d=8 ap_gather). Scalar ring: the
                    # sync ring is poisoned by mats buffer-wait stalls.
                    xTall = wp.tile([128, BT, KT_D], F32, tag="xTall", bufs=1,
                                    name=f"xT_{rep}")
                    nc.scalar.dma_start(xTall[:, 0:BT // 2, :],
                                        featT_d[:, 0:BT // 2, :])
                    nc.scalar.dma_start(xTall[:, BT // 2:, :],
                                        featT_d[:, BT // 2:, :])

                if phase == "null":
                    break

                # mats prefetch: 2-deep ring on the dedicated sync HWDGE ring
                matj = []
                for j in range(K):
                    t = mp.tile([128, KT_D, D], BF16, tag="matj",
                                name=f"matj{j}_{rep}")
                    nc.sync.dma_start(t[:], mats_d[j])
                    matj.append(t)

                with tc.tile_wait_until(t_p1(rep)):
                    # ----- h = relu(W1 @ x^T + b1) in f32r; maskT = (h>0) ---
                    relu = pp.tile([128, KT_H, BT], F32, tag="relu",
                                   name=f"relu_{rep}")
                    maskT = pp.tile([128, KT_H, BT], BF16, tag="maskT",
                                    name=f"maskT_{rep}")
                    for ns in range(NS):
                        nsl = slice(ns * 512, (ns + 1) * 512)
                        hacc = [psP.tile([128, 512], F32, tag=("B1", "B2")[ms],
                                         bufs=(2, 1)[ms], name=f"h{ms}_{ns}_{rep}")
                                for ms in range(KT_H)]
                        for kt in range(KT_D):
                            xr = wp.tile([128, 512], F32R, tag="xr",
                                         name=f"xr{kt}_{ns}_{rep}")
                            nc.scalar.copy(xr[:], xTall[:, nsl, kt])
                            for ms in range(KT_H):
                                nc.tensor.matmul(
                                    hacc[ms][:],
                                    w1tr[:, kt, ms * 128:(ms + 1) * 128],
                                    xr[:], start=(kt == 0), stop=(kt == KT_D - 1))
                        for ms in range(KT_H):
                            nc.scalar.activation(relu[:, ms, nsl], hacc[ms][:],
                                                 ACTF.Relu, bias=b1t[:, ms:ms + 1],
                                                 scale=1.0)
                            nc.vector.tensor_scalar(maskT[:, ms, nsl],
                                                    relu[:, ms, nsl],
                                                    0.0, None, ALU.is_gt)

                    # ----- u2T via transposed stage-1 + PE selector-ones
                    # reduce (emitted before coeffs so the collective input is
                    # ready as early as possible)
                    u2T = pp.tile([K, BT], F32, tag="u2T", name=f"u2T_{rep}")
                    for ns in range(NS):
                        nsl = slice(ns * 512, (ns + 1) * 512)
                        u2ps = psP.tile([K, 512], F32, tag="small", bufs=2,
                                        name=f"u2ps{ns}_{rep}")
                        nmm = 0
                        for k in range(K):
                            pair, half = divmod(k, 2)
                            hs = half * H
                            prods = []
                            for ms in range(KT_H):
                                xkt = psP.tile([128, 512], F32,
                                               tag=("B1", "B2")[k % 2],
                                               bufs=(2, 1)[k % 2],
                                               name=f"xkt{k}_{ms}_{ns}_{rep}")
                                for kt in range(KT_H):
                                    nc.tensor.matmul(
                                        xkt[:],
                                        g2all[:, kt, pair,
                                              hs + ms * 128:hs + (ms + 1) * 128],
                                        maskT[:, kt, nsl],
                                        start=(kt == 0), stop=(kt == KT_H - 1))
                                prod = wp.tile([128, 512], BF16, tag="prod",
                                               bufs=2,
                                               name=f"prod{k}_{ms}_{ns}_{rep}")
                                nc.vector.tensor_tensor(prod[:], xkt[:],
                                                        maskT[:, ms, nsl],
                                                        ALU.mult)
                                prods.append(prod)
                            for ms in range(KT_H):
                                nc.tensor.matmul(u2ps[:], onesel[:, k, :],
                                                 prods[ms][:],
                                                 start=(nmm == 0),
                                                 stop=(nmm == 2 * K - 1))
                                nmm += 1
                        nc.vector.tensor_copy(u2T[:, nsl], u2ps[:])

                    # ----- AllReduce max (high priority: issue the collective
                    # the moment u2T is done so peer skew starts draining)
                    with tc.high_priority():
                        lmax = wp.tile([128, 1], F32, tag="lmax",
                                       name=f"lmax_{rep}")
                        nc.vector.memset(lmax[:], 0.0)
                        nc.vector.tensor_reduce(lmax[0:K, :], u2T[:], AX.X,
                                                ALU.max)
                        nc.gpsimd.dma_start(cc_in[:], lmax[:])
                        if not sim:
                            nc.gpsimd.collective_compute(
                                "AllReduce", ALU.max,
                                replica_groups=[list(range(N_CORES))],
                                ins=[cc_in[:]], outs=[cc_out[:]])
                        gmax_col = wp.tile([1, 128], F32, tag="gmax_col",
                                           name=f"gmax_{rep}")
                        nc.gpsimd.dma_start(gmax_col[:],
                                            cc_in[:] if sim else cc_out[:])

                    # ----- coeffsT = W2 @ relu + b2 (fp32, T layout [K, BT]),
                    # only needed at gating time -----
                    coeffsT = pp.tile([K, BT], F32, tag="coeffsT",
                                      name=f"coeffsT_{rep}")
                    for ns in range(NS):
                        nsl = slice(ns * 512, (ns + 1) * 512)
                        cps = psP.tile([K, 512], F32, tag="small", bufs=2,
                                       name=f"cps{ns}_{rep}")
                        for kt in range(KT_H):
                            nc.tensor.matmul(cps[:], w2t[:, kt, :],
                                             relu[:, kt, nsl],
                                             start=(kt == 0),
                                             stop=(kt == KT_H - 1))
                        nc.scalar.activation(coeffsT[:, nsl], cps[:],
                                             ACTF.Identity, bias=b2col[:],
                                             scale=1.0)

                with tc.tile_wait_until(t_mach(rep)):
                    m2 = wp.tile([1, 1], F32, tag="m2", name=f"m2_{rep}")
                    nc.vector.tensor_reduce(m2[:], gmax_col[:], AX.X, ALU.max)
                    sq = wp.tile([1, 1], F32, tag="sqm2", name=f"sq_{rep}")
                    nc.scalar.activation(sq[:], m2[:], ACTF.Sqrt)
                    rs = wp.tile([1, 1], F32, tag="rsq", name=f"rs_{rep}")
                    nc.vector.reciprocal(rs[:], sq[:])
                    s11 = wp.tile([1, 1], F32, tag="s11", name=f"s11_{rep}")
                    nc.vector.tensor_tensor(s11[:], rs[:], scal[:, 1:2], ALU.mult)
                    sbc = wp.tile([128, 1], F32, tag="sbc", name=f"sbc_{rep}")
                    nc.gpsimd.partition_broadcast(sbc[:], s11[:])
                    basebc = wp.tile([128, 1], F32, tag="basebc",
                                     name=f"basebc_{rep}")
                    nc.gpsimd.partition_broadcast(basebc[:], scal[:, 0:1])

                    # ----- thresholds + gating (row-wise in T layout) -----
                    sqrT = wp.tile([K, BT], F32, tag="t8", name=f"sqrT_{rep}")
                    nc.scalar.activation(sqrT[:], u2T[:], ACTF.Sqrt)
                    thT = wp.tile([K, BT], F32, tag="t8", name=f"thT_{rep}")
                    nc.scalar.activation(thT[:], sqrT[:], ACTF.Identity,
                                         bias=basebc[0:K, :], scale=sbc[0:K, :])
                    absT = wp.tile([K, BT], F32, tag="t8", name=f"absT_{rep}")
                    nc.scalar.activation(absT[:], coeffsT[:], ACTF.Abs)
                    keepT = wp.tile([K, BT], F32, tag="keepT", name=f"keepT_{rep}")
                    nc.vector.tensor_tensor(keepT[:], absT[:], thT[:], ALU.is_ge)
                    gatedT16 = pp.tile([16, BT], F32, tag="gatedT16",
                                       name=f"gatedT16_{rep}")
                    nc.vector.memset(gatedT16[:], 0.0)
                    nc.vector.tensor_tensor(gatedT16[0:K, :], coeffsT[:],
                                            keepT[:], ALU.mult)
                    if debug_outs and rep == repeat - 1:
                        nc.gpsimd.dma_start(dbg_gated_d[:], gatedT16[0:K, :])
                        nc.gpsimd.dma_start(dbg_u2_d[:], u2T[:])

                    if upto == "gating":
                        continue

                    # unionK [128, NB] via PE transposes of keepT chunks
                    unionK = wp.tile([128, NB], F32, tag="unionK",
                                     name=f"unionK_{rep}")
                    for bt in range(NB):
                        tps = psP.tile([128, K], F32, tag="small", bufs=2,
                                       name=f"tpsu{bt}_{rep}")
                        nc.tensor.transpose(tps[:],
                                            keepT[:, bt * 128:(bt + 1) * 128],
                                            ident[0:K, 0:K])
                        nc.vector.tensor_reduce(unionK[:, bt:bt + 1], tps[:],
                                                AX.X, ALU.max)

                    # --- index machinery ---
                    u16 = wp.tile([16, NB, 8], F32, tag="u16", name=f"u16_{rep}")
                    for g in range(8):
                        ring = nc.scalar if g % 2 == 0 else nc.gpsimd
                        ring.dma_start(u16[:, :, g:g + 1],
                                       unionK[g * 16:(g + 1) * 16, :])
                    cand = wp.tile([16, BT // 16], F32, tag="cand",
                                   name=f"cand_{rep}")
                    # u16 layout: [p16, g, bt] maps to b = bt*128 + g*16 + p16;
                    # iota wrapped layout needs b = f*16 + p16, f = bt*8 + g.
                    u16flat = u16[:].rearrange("p b g -> p (b g)")
                    nc.vector.tensor_tensor(cand[:], iota16[:], u16flat, ALU.mult)
                    um1 = wp.tile([16, BT // 16], F32, tag="um1", name=f"um1_{rep}")
                    nc.vector.tensor_scalar(um1[:], u16flat, 1.0, None,
                                            ALU.subtract)
                    nc.vector.tensor_tensor(cand[:], cand[:], um1[:], ALU.add)
                    idxf = wp.tile([16, WF], F32, tag="idxf", name=f"idxf_{rep}")
                    cnt = wp.tile([1, 1], mybir.dt.uint32, tag="cnt",
                                  name=f"cnt_{rep}")
                    nc.gpsimd.sparse_gather(idxf[:], cand[:], num_found=cnt[:])
                    # mask junk tail: idx = idx if pos < cnt else -1
                    cntf = wp.tile([1, 1], F32, tag="cntf", name=f"cntf_{rep}")
                    nc.vector.tensor_copy(cntf[:], cnt[:])
                    cntbc = wp.tile([16, 1], F32, tag="cntbc", name=f"cntbc_{rep}")
                    nc.gpsimd.partition_broadcast(cntbc[:], cntf[:])
                    posok = wp.tile([16, WF], F32, tag="posok", name=f"posok_{rep}")
                    nc.vector.tensor_scalar(posok[:], positer[:], cntbc[:], None,
                                            ALU.is_lt)
                    nc.vector.tensor_tensor(idxf[:], idxf[:], posok[:], ALU.mult)
                    pm1 = wp.tile([16, WF], F32, tag="pm1", name=f"pm1_{rep}")
                    nc.vector.tensor_scalar(pm1[:], posok[:], 1.0, None,
                                            ALU.subtract)
                    nc.vector.tensor_tensor(idxf[:], idxf[:], pm1[:], ALU.add)
                    nc.gpsimd.dma_start(idxout_d[:], idxf[:])
                    idxf0 = wp.tile([16, WF], F32, tag="idxf0", name=f"idxf0_{rep}")
                    nc.vector.tensor_scalar(idxf0[:], idxf[:], 0.0, None, ALU.max)
                    idx16 = wp.tile([16, WF], I16, tag="idx16", name=f"idx16_{rep}")
                    nc.vector.tensor_copy(idx16[:], idxf0[:])
                    idxrep = wp.tile([128, WF], I16, tag="idxrep",
                                     name=f"idxrep_{rep}")
                    for g in range(8):
                        ring = nc.scalar if g % 2 == 0 else nc.gpsimd
                        ring.dma_start(idxrep[g * 16:(g + 1) * 16, :], idx16[:])

                    # gatedTc: compact gated rows; broadcast each row to all
                    # 128 partitions in bf16 (stage at partition 0 first —
                    # this concourse requires partition-0 sources)
                    gatedTc = wp.tile([16, CW], F32, tag="gatedTc", bufs=1,
                                      name=f"gatedTc_{rep}")
                    nc.gpsimd.ap_gather(gatedTc[:], gatedT16[:], idx16[:, 0:WFC],
                                        channels=16, num_elems=BT, d=1,
                                        num_idxs=CW)
                    gatedTcb = wp.tile([K, CW], BF16, tag="gatedTcb",
                                       name=f"gatedTcb_{rep}")
                    nc.vector.tensor_copy(gatedTcb[:], gatedTc[0:K, :])
                    gflat = wp.tile([1, K * CW], BF16, tag="gflat", bufs=1,
                                    name=f"gflat_{rep}")
                    for j in range(K):
                        ring = nc.scalar if j % 2 == 0 else nc.gpsimd
                        ring.dma_start(gflat[0:1, j * CW:(j + 1) * CW],
                                       gatedTcb[j:j + 1, :])
                    gball = wp.tile([128, K, CW], BF16, tag="gball", bufs=1,
                                    name=f"gball_{rep}")
                    for j in range(K):
                        nc.gpsimd.partition_broadcast(
                            gball[:, j, :], gflat[0:1, j * CW:(j + 1) * CW])

                    # gather compacted ORIGINAL x straight from DRAM:
                    # one transposing dma_gather of CWG 2KB rows (bf16)
                    xgb = wp.tile([128, KT_D, CWG], BF16, tag="xgb", bufs=2,
                                  name=f"xgb_{rep}")
                    nc.gpsimd.dma_gather(xgb[:], featB_d[:], idxrep[:],
                                         CWG, CWG, D, transpose=True)

                # ----- compact combination loop, ALL 8 steps; running x in
                # SBUF f32 (xc), updates accumulate per step via DVE/gpsimd
                # adds; PSUM holds only the transient per-(j,m) matmul result.
                with tc.tile_wait_until(t_comp(rep)):
                    xc = wp.tile([128, KT_D, CW], F32, tag="xc", bufs=1,
                                 name=f"xc_{rep}")

                    def rc_tile(i, nm):
                        return wp.tile([128, KT_D, CW], BF16,
                                       tag=("rcA", "rcB")[i % 2], bufs=1,
                                       name=nm)
                    rc = rc_tile(0, f"rc0_{rep}")
                    for m in range(KT_D):
                        nc.vector.tensor_tensor(
                            rc[:, m, :], xgb[:, m, 0:CW], gball[:, 0, :],
                            ALU.mult)
                    for j in range(K):
                        rcn = rc_tile(j + 1, f"rc{j+1}_{rep}") if j + 1 < K \
                            else None
                        for m in range(KT_D):
                            ps = psP.tile([128, CW], F32, tag="A", bufs=3,
                                          name=f"cps{j}_{m}_{rep}")
                            for kt in range(KT_D):
                                nc.tensor.matmul(
                                    ps[:], matj[j][:, kt, m * 128:(m + 1) * 128],
                                    rc[:, kt, :],
                                    start=(kt == 0), stop=(kt == KT_D - 1))
                            if j == 0:
                                nc.vector.tensor_tensor(
                                    xc[:, m, :], xgb[:, m, 0:CW], ps[:], ALU.add)
                            else:
                                # PSUM reads must stay off gpsimd
                                nc.vector.tensor_tensor(
                                    xc[:, m, :], xc[:, m, :], ps[:], ALU.add)
                            if rcn is not None:
                                # SBUF-only multiply: alternate engines to
                                # balance DVE/gpsimd load
                                eng = nc.vector if m % 2 == 0 else nc.gpsimd
                                eng.tensor_tensor(
                                    rcn[:, m, :], xc[:, m, :],
                                    gball[:, j + 1, :], ALU.mult)
                        rc = rcn

                    if phase != "null":
                        nc.gpsimd.dma_start(outc_d[:], xc[:])

    nc.finalize()
    return nc


def make_inputs(features_shard_T, W1, b1, W2, b2, task_mats, base_threshold, beta,
                CW=CW_DEFAULT):
    import ml_dtypes
    BT = features_shard_T.shape[1]
    featT = np.ascontiguousarray(
        features_shard_T.reshape(KT_D, 128, BT).transpose(1, 2, 0))
    w1t = np.ascontiguousarray(W1.T.reshape(KT_D, 128, H).transpose(1, 0, 2))
    w2t = np.ascontiguousarray(W2.T.reshape(KT_H, 128, K).transpose(1, 0, 2))
    b1t = np.ascontiguousarray(b1.reshape(KT_H, 128).T)
    base_sp = np.log1p(np.exp(np.float32(base_threshold[0]))).astype(np.float32)
    rbeta = np.maximum(np.float32(beta[0]), 0).astype(np.float32)
    scal = np.array([[base_sp, base_sp * rbeta, 0, 0]], np.float32)
    mats = np.ascontiguousarray(
        task_mats.reshape(K, KT_D, 128, D).transpose(0, 2, 1, 3)).astype(ml_dtypes.bfloat16)
    iota16 = (np.arange(BT).reshape(BT // 16, 16).T).astype(np.float32)
    positer = (np.arange(512).reshape(32, 16).T).astype(np.float32)
    # host-side weight-only precompute: G2 pairs in the device layout
    # g2p[p, kt, pair, half*H + h'] = W2[k,h] * G[h,h'] * W2[k,h']
    # with h = kt*128 + p, k = 2*pair + half
    G = (W1.astype(np.float32) @ W1.T.astype(np.float32))
    g2p = np.empty((128, KT_H, K // 2, 2 * H), np.float32)
    for pair in range(K // 2):
        for half in range(2):
            k = 2 * pair + half
            G2k = W2[k][:, None] * G * W2[k][None, :]
            for kt in range(KT_H):
                g2p[:, kt, pair, half * H:(half + 1) * H] = \
                    G2k[kt * 128:(kt + 1) * 128, :]
    return {
        "iota16": np.ascontiguousarray(iota16),
        "positer": np.ascontiguousarray(positer),
        "featT": featT.astype(np.float32),
        "featB": np.ascontiguousarray(features_shard_T.T).astype(ml_dtypes.bfloat16),
        "w1t": w1t.astype(np.float32),
        "w2t": w2t.astype(np.float32),
        "g2p": g2p.astype(ml_dtypes.bfloat16),
        "b1t": b1t.astype(np.float32),
        "b2col": b2.reshape(K, 1).astype(np.float32),
        "onesel": np.broadcast_to(np.eye(K, dtype=np.float32)[None, :, :],
                                  (128, K, K)).astype(ml_dtypes.bfloat16),
        "ident": np.eye(128, dtype=np.float32),
        "scal": scal,
        "mats": mats,
    }




_CACHE = {}


def _host_reference(features, W1, b1, W2, b2, task_mats, base_threshold, beta):
    """Pure-numpy fallback (only for pathological gating densities)."""
    f = features.astype(np.float64)
    h = f @ W1.T.astype(np.float64) + b1.astype(np.float64)
    relu_h = np.maximum(h, 0.0)
    coeffs = relu_h @ W2.T.astype(np.float64) + b2.astype(np.float64)
    mask = (h > 0).astype(np.float64)
    G = W1.astype(np.float64) @ W1.T.astype(np.float64)
    u2 = np.zeros((f.shape[0], K))
    for k in range(K):
        V = mask * W2[k].astype(np.float64)[None, :]
        u2[:, k] = np.sum((V @ G) * V, axis=1)
    unc = np.sqrt(u2)
    mx = unc.max()
    if mx > 0:
        unc = unc / mx
    base = np.log1p(np.exp(np.float64(base_threshold.reshape(-1)[0])))
    th = base * (1.0 + max(float(beta.reshape(-1)[0]), 0.0) * unc)
    gated = np.where(np.abs(coeffs) < th, 0.0, coeffs)
    x = f.copy()
    for j in range(K):
        x = x + gated[:, j:j + 1] * (x @ task_mats[j].astype(np.float64))
    return x.astype(np.float32)


def _union_counts(features, W1, b1, W2, b2, base_threshold, beta):
    """Approximate per-core union-active counts (sizing check only)."""
    f = features.astype(np.float32)
    h = f @ W1.T + b1
    coeffs = np.maximum(h, 0) @ W2.T + b2
    mask = (h > 0).astype(np.float32)
    G = (W1 @ W1.T).astype(np.float32)
    u2 = np.zeros((f.shape[0], K), np.float32)
    for k in range(K):
        V = mask * W2[k][None, :]
        u2[:, k] = np.sum((V @ G) * V, axis=1)
    unc = np.sqrt(u2)
    mx = unc.max()
    if mx > 0:
        unc = unc / mx
    base = np.log1p(np.exp(np.float32(base_threshold.reshape(-1)[0])))
    th = base * (1 + max(float(beta.reshape(-1)[0]), 0.0) * unc)
    active = (np.abs(coeffs) >= th).any(axis=1)
    BT = f.shape[0] // N_CORES
    return [int(active[c * BT:(c + 1) * BT].sum()) for c in range(N_CORES)]


def kernel(features, W1, b1, W2, b2, task_mats, proj_W, base_threshold, beta,
           **_unused):
    from concourse.bass_utils import run_bass_kernel_spmd

    features = np.asarray(features, dtype=np.float32)
    W1 = np.asarray(W1, np.float32)
    b1 = np.asarray(b1, np.float32)
    W2 = np.asarray(W2, np.float32)
    b2 = np.asarray(b2, np.float32)
    task_mats = np.asarray(task_mats, np.float32)
    base_threshold = np.asarray(base_threshold, np.float32)
    beta = np.asarray(beta, np.float32)
    B = features.shape[0]
    BT = B // N_CORES
    CW = CW_DEFAULT

    # sizing check: the compact loop holds up to CW active columns per core.
    # device/host gate decisions can differ by a couple of borderline samples,
    # so require a safety margin; otherwise fall back to exact host compute.
    counts = _union_counts(features, W1, b1, W2, b2, base_threshold, beta)
    x = None
    if max(counts) <= CW - 24 and B % N_CORES == 0:
        try:
            key = ("nc", BT, CW)
            if key not in _CACHE:
                _CACHE[key] = build(BT=BT, CW=CW)
            nc = _CACHE[key]
            in_maps = []
            for c in range(N_CORES):
                shard_T = np.ascontiguousarray(features[c * BT:(c + 1) * BT].T)
                in_maps.append(make_inputs(shard_T, W1, b1, W2, b2, task_mats,
                                           base_threshold, beta, CW=CW))
            res = run_bass_kernel_spmd(nc, in_maps, core_ids=list(range(N_CORES)))
            outs = []
            for c in range(N_CORES):
                r = res.results[c]
                idx1d = r["idxout"].T.reshape(-1)[:CW]  # unwrap i = f*16 + p
                valid = idx1d >= 0
                idxs = idx1d[valid].astype(np.int64)
                xcT = r["outc"].transpose(1, 0, 2).reshape(D, CW)
                xfull = features[c * BT:(c + 1) * BT].copy()
                xfull[idxs, :] = xcT[:, valid].T
                outs.append(xfull)
            x = np.concatenate(outs, axis=0)
        except Exception as e:
            import traceback
            print(f"kernel: device path failed ({e!r}); falling back to host",
                  file=sys.stderr)
            traceback.print_exc()
            x = None
    if x is None:
        x = _host_reference(features, W1, b1, W2, b2, task_mats,
                            base_threshold, beta)

    proj_W = np.asarray(proj_W, np.float32)
    if not (proj_W.shape == (D, D) and
            np.array_equal(proj_W, np.eye(D, dtype=proj_W.dtype))):
        x = (x @ proj_W.T).astype(np.float32)
    return np.ascontiguousarray(x.astype(np.float32))


# revision 34
# speedup vs baseline: 1.3476x; 1.1116x over previous
"""AdaptiveGatingMetaNet on 8 Trainium2 NeuronCores (Bass/Tile SPMD).

Data-parallel over batch (1024 rows/core); weights replicated. Per core:
  - meta-net h in float32r (1 cyc/row PE, ~1.5e-4 rel err — gate-flip safe),
    coeffs in fp32
  - uncertainty via the Gram trick in bf16 (host-verified: zero gate flips):
    unc^2[b,k] = m_b^T diag(w2k) G diag(w2k) m_b with G = W1 W1^T
  - global max via one 512B AllReduce(max)
  - combination: ALL 8 steps run on the compacted union of active batch
    columns (CW=416, measured max 389); x is gathered once from the original
    features in a d=8-blocked layout (one 4KB-descriptor ap_gather), and the
    running x accumulates in SBUF f32 (PSUM stays free for the next rep's
    meta-net, enabling cross-rep software pipelining via wait_until tiers)
  - task matrices (16MB bf16) stream on a dedicated sync DGE ring
"""
import sys
sys.path.insert(0, "/opt/trn_rl_repo")
import numpy as np
from concourse import bass, bacc, tile, mybir

F32 = mybir.dt.float32
F32R = mybir.dt.float32r
BF16 = mybir.dt.bfloat16
I16 = mybir.dt.int16
AX = mybir.AxisListType
ALU = mybir.AluOpType
ACTF = mybir.ActivationFunctionType

D = 1024
H = 256
K = 8
KT_D = D // 128
KT_H = H // 128
N_CORES = 8
CW_DEFAULT = 416


def build(BT=1024, debug_outs=False, phase="full", repeat=1, CW=CW_DEFAULT,
          upto="full", sim=False):
    """Per-core SPMD kernel.

    phase: "full" | "null" (null = passthrough, for dispatch calibration)
    repeat: run the whole computation N times (timing amortization)
    """
    NB = BT // 128
    NS = BT // 512
    CWG = 512
    WF = CWG // 16
    WFC = CW // 16
    nc = bacc.Bacc("TRN2", target_bir_lowering=False, debug=False,
                   num_devices=N_CORES)

    featT_d = nc.dram_tensor("featT", [128, BT, KT_D], F32, kind="ExternalInput")
    featB_d = nc.dram_tensor("featB", [BT, D], BF16, kind="ExternalInput")
    w1t_d = nc.dram_tensor("w1t", [128, KT_D, H], F32, kind="ExternalInput")
    w2t_d = nc.dram_tensor("w2t", [128, KT_H, K], F32, kind="ExternalInput")
    g2p_d = nc.dram_tensor("g2p", [128, KT_H, K // 2, 2 * H], BF16,
                           kind="ExternalInput")
    b1t_d = nc.dram_tensor("b1t", [128, KT_H], F32, kind="ExternalInput")
    b2col_d = nc.dram_tensor("b2col", [K, 1], F32, kind="ExternalInput")
    onesel_d = nc.dram_tensor("onesel", [128, K, 8], BF16, kind="ExternalInput")
    ident_d = nc.dram_tensor("ident", [128, 128], F32, kind="ExternalInput")
    scal_d = nc.dram_tensor("scal", [1, 4], F32, kind="ExternalInput")
    onesrow_d = nc.dram_tensor("onesrow", [K, K, 128], F32, kind="ExternalInput")
    mats_d = nc.dram_tensor("mats", [K, 128, KT_D, D], BF16, kind="ExternalInput")
    iota16_d = nc.dram_tensor("iota16", [16, BT // 16], F32, kind="ExternalInput")
    positer_d = nc.dram_tensor("positer", [16, WF], F32, kind="ExternalInput")
    outc_d = nc.dram_tensor("outc", [128, KT_D, CW], F32, kind="ExternalOutput")
    idxout_d = nc.dram_tensor("idxout", [16, WF], F32, kind="ExternalOutput")
    if debug_outs:
        dbg_gated_d = nc.dram_tensor("dbg_gated", [K, BT], F32, kind="ExternalOutput")
        dbg_u2_d = nc.dram_tensor("dbg_u2", [K, BT], F32, kind="ExternalOutput")

    cc_in = nc.dram_tensor("cc_in", [128], F32)
    cc_out = nc.dram_tensor("cc_out", [128], F32, addr_space="Shared")

    # wait_until tiers (logical ms): phase1 of rep r at r*P1; the gating-
    # dependent tail of rep r sorts after phase1 of reps r+1 AND r+2 so the
    # in-order PE fills the collective+machinery latency with the next reps'
    # meta-net matmuls (software pipelining, depth 2).
    P1 = 0.2
    t_p1 = lambda r: r * P1
    t_mach = lambda r: r * P1 + 0.35
    t_comp = lambda r: r * P1 + 0.5

    with tile.TileContext(nc) as tc:
        with (
            tc.tile_pool(name="persist", bufs=1) as pp,
            tc.tile_pool(name="work", bufs=2) as wp,
            tc.tile_pool(name="rmats", bufs=2) as mp,
            tc.tile_pool(name="psP", bufs=1, space="PSUM") as psP,
        ):
            # ----- constants (once): gpsimd/scalar rings; the sync ring is
            # reserved for the 16MB/rep mats stream -----
            w1t = pp.tile([128, KT_D, H], F32, tag="w1t")
            nc.gpsimd.dma_start(w1t[:], w1t_d[:])
            w2t = pp.tile([128, KT_H, K], F32, tag="w2t")
            nc.gpsimd.dma_start(w2t[:], w2t_d[:])
            b1t = pp.tile([128, KT_H], F32, tag="b1t")
            nc.gpsimd.dma_start(b1t[:], b1t_d[:])
            ident = pp.tile([128, 128], F32, tag="ident")
            nc.gpsimd.dma_start(ident[:], ident_d[:])
            scal = pp.tile([1, 4], F32, tag="scal")
            nc.gpsimd.dma_start(scal[:], scal_d[:])
            b2col = pp.tile([K, 1], F32, tag="b2col")
            nc.gpsimd.dma_start(b2col[:], b2col_d[:])
            onesel = pp.tile([128, K, 8], BF16, tag="onesel")
            nc.gpsimd.dma_start(onesel[:], onesel_d[:])
            onesrow = pp.tile([K, K, 128], F32, tag="onesrow")
            nc.gpsimd.dma_start(onesrow[:], onesrow_d[:])
            iota16 = pp.tile([16, BT // 16], F32, tag="iota16")
            nc.gpsimd.dma_start(iota16[:], iota16_d[:])
            positer = pp.tile([16, WF], F32, tag="positer")
            nc.gpsimd.dma_start(positer[:], positer_d[:])
            w1tr = pp.tile([128, KT_D, H], F32R, tag="w1tr")
            nc.vector.tensor_copy(w1tr[:], w1t[:])
            # G2 pairs (diag(w2k) W1W1^T diag(w2k)) are weight-only: computed
            # on the host, streamed in as one 2MB bf16 input
            g2all = pp.tile([128, KT_H, K // 2, 2 * H], BF16, tag="g2all")
            nc.scalar.dma_start(g2all[:], g2p_d[:])

            for rep in range(repeat):
                with tc.tile_wait_until(t_p1(rep)):
                    # ----- x load in [128, b, m] layout (m fastest, so the
                    # compact gather is ONE d=8 ap_gather). Scalar ring: the
                    # sync ring is poisoned by mats buffer-wait stalls.
                    xTall = wp.tile([128, BT, KT_D], F32, tag="xTall", bufs=1,
                                    name=f"xT_{rep}")
                    nc.scalar.dma_start(xTall[:, 0:BT // 2, :],
                                        featT_d[:, 0:BT // 2, :])
                    nc.scalar.dma_start(xTall[:, BT // 2:, :],
                                        featT_d[:, BT // 2:, :])

                if phase == "null":
                    break

                # mats prefetch: 2-deep ring on the dedicated sync HWDGE ring
                matj = []
                for j in range(K):
                    t = mp.tile([128, KT_D, D], BF16, tag="matj",
                                name=f"matj{j}_{rep}")
                    nc.sync.dma_start(t[:], mats_d[j])
                    matj.append(t)

                with tc.tile_wait_until(t_p1(rep)):
                    # ----- h = relu(W1 @ x^T + b1) in f32r; maskT = (h>0) ---
                    relu = pp.tile([128, KT_H, BT], F32, tag="relu",
                                   name=f"relu_{rep}")
                    maskT = pp.tile([128, KT_H, BT], BF16, tag="maskT",
                                    name=f"maskT_{rep}")
                    for ns in range(NS):
                        nsl = slice(ns * 512, (ns + 1) * 512)
                        hacc = [psP.tile([128, 512], F32, tag=("B1", "B2")[ms],
                                         bufs=(2, 1)[ms], name=f"h{ms}_{ns}_{rep}")
                                for ms in range(KT_H)]
                        for kt in range(KT_D):
                            xr = wp.tile([128, 512], F32R, tag="xr",
                                         name=f"xr{kt}_{ns}_{rep}")
                            nc.scalar.copy(xr[:], xTall[:, nsl, kt])
                            for ms in range(KT_H):
                                nc.tensor.matmul(
                                    hacc[ms][:],
                                    w1tr[:, kt, ms * 128:(ms + 1) * 128],
                                    xr[:], start=(kt == 0), stop=(kt == KT_D - 1))
                        for ms in range(KT_H):
                            nc.scalar.activation(relu[:, ms, nsl], hacc[ms][:],
                                                 ACTF.Relu, bias=b1t[:, ms:ms + 1],
                                                 scale=1.0)
                            nc.vector.tensor_scalar(maskT[:, ms, nsl],
                                                    relu[:, ms, nsl],
                                                    0.0, None, ALU.is_gt)

                    # ----- u2T via transposed stage-1 + PE selector-ones
                    # reduce (emitted before coeffs so the collective input is
                    # ready as early as possible)
                    u2T = pp.tile([K, BT], F32, tag="u2T", name=f"u2T_{rep}")
                    for ns in range(NS):
                        nsl = slice(ns * 512, (ns + 1) * 512)
                        u2ps = psP.tile([K, 512], F32, tag="small", bufs=2,
                                        name=f"u2ps{ns}_{rep}")
                        nmm = 0
                        for k in range(K):
                            pair, half = divmod(k, 2)
                            hs = half * H
                            prods = []
                            for ms in range(KT_H):
                                xkt = psP.tile([128, 512], F32,
                                               tag=("B1", "B2")[k % 2],
                                               bufs=(2, 1)[k % 2],
                                               name=f"xkt{k}_{ms}_{ns}_{rep}")
                                for kt in range(KT_H):
                                    nc.tensor.matmul(
                                        xkt[:],
                                        g2all[:, kt, pair,
                                              hs + ms * 128:hs + (ms + 1) * 128],
                                        maskT[:, kt, nsl],
                                        start=(kt == 0), stop=(kt == KT_H - 1))
                                prod = wp.tile([128, 512], BF16, tag="prod",
                                               bufs=2,
                                               name=f"prod{k}_{ms}_{ns}_{rep}")
                                nc.vector.tensor_tensor(prod[:], xkt[:],
                                                        maskT[:, ms, nsl],
                                                        ALU.mult)
                                prods.append(prod)
                            for ms in range(KT_H):
                                nc.tensor.matmul(u2ps[:], onesel[:, k, :],
                                                 prods[ms][:],
                                                 start=(nmm == 0),
                                                 stop=(nmm == 2 * K - 1))
                                nmm += 1
                        nc.vector.tensor_copy(u2T[:, nsl], u2ps[:])

                    # ----- AllReduce max (high priority: issue the collective
                    # the moment u2T is done so peer skew starts draining)
                    with tc.high_priority():
                        lmax = wp.tile([128, 1], F32, tag="lmax",
                                       name=f"lmax_{rep}")
                        nc.vector.memset(lmax[:], 0.0)
                        nc.vector.tensor_reduce(lmax[0:K, :], u2T[:], AX.X,
                                                ALU.max)
                        nc.gpsimd.dma_start(cc_in[:], lmax[:])
                        if not sim:
                            nc.gpsimd.collective_compute(
                                "AllReduce", ALU.max,
                                replica_groups=[list(range(N_CORES))],
                                ins=[cc_in[:]], outs=[cc_out[:]])
                        gmax_col = wp.tile([1, 128], F32, tag="gmax_col",
                                           name=f"gmax_{rep}")
                        nc.gpsimd.dma_start(gmax_col[:],
                                            cc_in[:] if sim else cc_out[:])

                    # ----- coeffsT = W2 @ relu + b2 (fp32, T layout [K, BT]),
                    # only needed at gating time -----
                    coeffsT = pp.tile([K, BT], F32, tag="coeffsT",
                                      name=f"coeffsT_{rep}")
                    for ns in range(NS):
                        nsl = slice(ns * 512, (ns + 1) * 512)
                        cps = psP.tile([K, 512], F32, tag="small", bufs=2,
                                       name=f"cps{ns}_{rep}")
                        for kt in range(KT_H):
                            nc.tensor.matmul(cps[:], w2t[:, kt, :],
                                             relu[:, kt, nsl],
                                             start=(kt == 0),
                                             stop=(kt == KT_H - 1))
                        nc.scalar.activation(coeffsT[:, nsl], cps[:],
                                             ACTF.Identity, bias=b2col[:],
                                             scale=1.0)

                with tc.tile_wait_until(t_mach(rep)):
                    m2 = wp.tile([1, 1], F32, tag="m2", name=f"m2_{rep}")
                    nc.vector.tensor_reduce(m2[:], gmax_col[:], AX.X, ALU.max)
                    sq = wp.tile([1, 1], F32, tag="sqm2", name=f"sq_{rep}")
                    nc.scalar.activation(sq[:], m2[:], ACTF.Sqrt)
                    rs = wp.tile([1, 1], F32, tag="rsq", name=f"rs_{rep}")
                    nc.vector.reciprocal(rs[:], sq[:])
                    s11 = wp.tile([1, 1], F32, tag="s11", name=f"s11_{rep}")
                    nc.vector.tensor_tensor(s11[:], rs[:], scal[:, 1:2], ALU.mult)
                    sbc = wp.tile([128, 1], F32, tag="sbc", name=f"sbc_{rep}")
                    nc.gpsimd.partition_broadcast(sbc[:], s11[:])
                    basebc = wp.tile([128, 1], F32, tag="basebc",
                                     name=f"basebc_{rep}")
                    nc.gpsimd.partition_broadcast(basebc[:], scal[:, 0:1])

                    # ----- thresholds + gating (row-wise in T layout) -----
                    sqrT = wp.tile([K, BT], F32, tag="t8", name=f"sqrT_{rep}")
                    nc.scalar.activation(sqrT[:], u2T[:], ACTF.Sqrt)
                    thT = wp.tile([K, BT], F32, tag="t8", name=f"thT_{rep}")
                    nc.scalar.activation(thT[:], sqrT[:], ACTF.Identity,
                                         bias=basebc[0:K, :], scale=sbc[0:K, :])
                    absT = wp.tile([K, BT], F32, tag="t8", name=f"absT_{rep}")
                    nc.scalar.activation(absT[:], coeffsT[:], ACTF.Abs)
                    keepT = wp.tile([K, BT], F32, tag="keepT", name=f"keepT_{rep}")
                    nc.vector.tensor_tensor(keepT[:], absT[:], thT[:], ALU.is_ge)
                    gatedT16 = pp.tile([16, BT], F32, tag="gatedT16",
                                       name=f"gatedT16_{rep}")
                    nc.vector.memset(gatedT16[:], 0.0)
                    nc.vector.tensor_tensor(gatedT16[0:K, :], coeffsT[:],
                                            keepT[:], ALU.mult)
                    if debug_outs and rep == repeat - 1:
                        nc.gpsimd.dma_start(dbg_gated_d[:], gatedT16[0:K, :])
                        nc.gpsimd.dma_start(dbg_u2_d[:], u2T[:])

                    if upto == "gating":
                        continue

                    # unionK [128, NB] via PE transposes of keepT chunks
                    unionK = wp.tile([128, NB], F32, tag="unionK",
                                     name=f"unionK_{rep}")
                    for bt in range(NB):
                        tps = psP.tile([128, K], F32, tag="small", bufs=2,
                                       name=f"tpsu{bt}_{rep}")
                        nc.tensor.transpose(tps[:],
                                            keepT[:, bt * 128:(bt + 1) * 128],
                                            ident[0:K, 0:K])
                        nc.vector.tensor_reduce(unionK[:, bt:bt + 1], tps[:],
                                                AX.X, ALU.max)

                    # --- index machinery ---
                    u16 = wp.tile([16, NB, 8], F32, tag="u16", name=f"u16_{rep}")
                    for g in range(8):
                        ring = nc.scalar if g % 2 == 0 else nc.gpsimd
                        ring.dma_start(u16[:, :, g:g + 1],
                                       unionK[g * 16:(g + 1) * 16, :])
                    cand = wp.tile([16, BT // 16], F32, tag="cand",
                                   name=f"cand_{rep}")
                    # u16 layout: [p16, g, bt] maps to b = bt*128 + g*16 + p16;
                    # iota wrapped layout needs b = f*16 + p16, f = bt*8 + g.
                    u16flat = u16[:].rearrange("p b g -> p (b g)")
                    nc.vector.tensor_tensor(cand[:], iota16[:], u16flat, ALU.mult)
                    um1 = wp.tile([16, BT // 16], F32, tag="um1", name=f"um1_{rep}")
                    nc.vector.tensor_scalar(um1[:], u16flat, 1.0, None,
                                            ALU.subtract)
                    nc.vector.tensor_tensor(cand[:], cand[:], um1[:], ALU.add)
                    idxf = wp.tile([16, WF], F32, tag="idxf", name=f"idxf_{rep}")
                    cnt = wp.tile([1, 1], mybir.dt.uint32, tag="cnt",
                                  name=f"cnt_{rep}")
                    nc.gpsimd.sparse_gather(idxf[:], cand[:], num_found=cnt[:])
                    # mask junk tail: idx = idx if pos < cnt else -1
                    cntf = wp.tile([1, 1], F32, tag="cntf", name=f"cntf_{rep}")
                    nc.vector.tensor_copy(cntf[:], cnt[:])
                    cntbc = wp.tile([16, 1], F32, tag="cntbc", name=f"cntbc_{rep}")
                    nc.gpsimd.partition_broadcast(cntbc[:], cntf[:])
                    posok = wp.tile([16, WF], F32, tag="posok", name=f"posok_{rep}")
                    nc.vector.tensor_scalar(posok[:], positer[:], cntbc[:], None,
                                            ALU.is_lt)
                    nc.vector.tensor_tensor(idxf[:], idxf[:], posok[:], ALU.mult)
                    pm1 = wp.tile([16, WF], F32, tag="pm1", name=f"pm1_{rep}")
                    nc.vector.tensor_scalar(pm1[:], posok[:], 1.0, None,
                                            ALU.subtract)
                    nc.vector.tensor_tensor(idxf[:], idxf[:], pm1[:], ALU.add)
                    nc.gpsimd.dma_start(idxout_d[:], idxf[:])
                    idxf0 = wp.tile([16, WF], F32, tag="idxf0", name=f"idxf0_{rep}")
                    nc.vector.tensor_scalar(idxf0[:], idxf[:], 0.0, None, ALU.max)
                    idx16 = wp.tile([16, WF], I16, tag="idx16", name=f"idx16_{rep}")
                    nc.vector.tensor_copy(idx16[:], idxf0[:])
                    idxrep = wp.tile([128, WF], I16, tag="idxrep",
                                     name=f"idxrep_{rep}")
                    for g in range(8):
                        ring = nc.scalar if g % 2 == 0 else nc.gpsimd
                        ring.dma_start(idxrep[g * 16:(g + 1) * 16, :], idx16[:])

                    # gatedTc: compact gated rows; broadcast each row to all
                    # 128 partitions in bf16 (stage at partition 0 first —
                    # this concourse requires partition-0 sources)
                    gatedTc = wp.tile([16, CW], F32, tag="gatedTc", bufs=1,
                                      name=f"gatedTc_{rep}")
                    nc.gpsimd.ap_gather(gatedTc[:], gatedT16[:], idx16[:, 0:WFC],
                                        channels=16, num_elems=BT, d=1,
                                        num_idxs=CW)
                    gball = wp.tile([128, K, CW], BF16, tag="gball", bufs=1,
                                    name=f"gball_{rep}")
                    for j in range(K):
                        gbps = psP.tile([128, CW], F32, tag="small", bufs=2,
                                        name=f"gbps{j}_{rep}")
                        nc.tensor.matmul(gbps[:], onesrow[:, j, :],
                                         gatedTc[0:K, :], start=True, stop=True)
                        nc.vector.tensor_copy(gball[:, j, :], gbps[:])

                    # gather compacted ORIGINAL x straight from DRAM:
                    # one transposing dma_gather of CWG 2KB rows (bf16)
                    xgb = wp.tile([128, KT_D, CWG], BF16, tag="xgb", bufs=2,
                                  name=f"xgb_{rep}")
                    nc.gpsimd.dma_gather(xgb[:], featB_d[:], idxrep[:],
                                         CWG, CWG, D, transpose=True)

                # ----- compact combination loop, ALL 8 steps; running x in
                # SBUF f32 (xc), updates accumulate per step via DVE/gpsimd
                # adds; PSUM holds only the transient per-(j,m) matmul result.
                with tc.tile_wait_until(t_comp(rep)):
                    xc = wp.tile([128, KT_D, CW], F32, tag="xc", bufs=1,
                                 name=f"xc_{rep}")

                    def rc_tile(i, nm):
                        return wp.tile([128, KT_D, CW], BF16,
                                       tag=("rcA", "rcB")[i % 2], bufs=1,
                                       name=nm)
                    rc = rc_tile(0, f"rc0_{rep}")
                    for m in range(KT_D):
                        nc.vector.tensor_tensor(
                            rc[:, m, :], xgb[:, m, 0:CW], gball[:, 0, :],
                            ALU.mult)
                    for j in range(K):
                        rcn = rc_tile(j + 1, f"rc{j+1}_{rep}") if j + 1 < K \
                            else None
                        for m in range(KT_D):
                            ps = psP.tile([128, CW], F32, tag="A", bufs=3,
                                          name=f"cps{j}_{m}_{rep}")
                            for kt in range(KT_D):
                                nc.tensor.matmul(
                                    ps[:], matj[j][:, kt, m * 128:(m + 1) * 128],
                                    rc[:, kt, :],
                                    start=(kt == 0), stop=(kt == KT_D - 1))
                            if j == 0:
                                nc.vector.tensor_tensor(
                                    xc[:, m, :], xgb[:, m, 0:CW], ps[:], ALU.add)
                            else:
                                # PSUM reads must stay off gpsimd
                                nc.vector.tensor_tensor(
                                    xc[:, m, :], xc[:, m, :], ps[:], ALU.add)
                            if rcn is not None:
                                nc.vector.tensor_tensor(
                                    rcn[:, m, :], xc[:, m, :],
                                    gball[:, j + 1, :], ALU.mult)
                        rc = rcn

                    if phase != "null":
                        nc.gpsimd.dma_start(outc_d[:], xc[:])

    nc.finalize()
    return nc


def make_inputs(features_shard_T, W1, b1, W2, b2, task_mats, base_threshold, beta,
                CW=CW_DEFAULT):
    import ml_dtypes
    BT = features_shard_T.shape[1]
    featT = np.ascontiguousarray(
        features_shard_T.reshape(KT_D, 128, BT).transpose(1, 2, 0))
    w1t = np.ascontiguousarray(W1.T.reshape(KT_D, 128, H).transpose(1, 0, 2))
    w2t = np.ascontiguousarray(W2.T.reshape(KT_H, 128, K).transpose(1, 0, 2))
    b1t = np.ascontiguousarray(b1.reshape(KT_H, 128).T)
    base_sp = np.log1p(np.exp(np.float32(base_threshold[0]))).astype(np.float32)
    rbeta = np.maximum(np.float32(beta[0]), 0).astype(np.float32)
    scal = np.array([[base_sp, base_sp * rbeta, 0, 0]], np.float32)
    mats = np.ascontiguousarray(
        task_mats.reshape(K, KT_D, 128, D).transpose(0, 2, 1, 3)).astype(ml_dtypes.bfloat16)
    iota16 = (np.arange(BT).reshape(BT // 16, 16).T).astype(np.float32)
    positer = (np.arange(512).reshape(32, 16).T).astype(np.float32)
    # host-side weight-only precompute: G2 pairs in the device layout
    # g2p[p, kt, pair, half*H + h'] = W2[k,h] * G[h,h'] * W2[k,h']
    # with h = kt*128 + p, k = 2*pair + half
    G = (W1.astype(np.float32) @ W1.T.astype(np.float32))
    g2p = np.empty((128, KT_H, K // 2, 2 * H), np.float32)
    for pair in range(K // 2):
        for half in range(2):
            k = 2 * pair + half
            G2k = W2[k][:, None] * G * W2[k][None, :]
            for kt in range(KT_H):
                g2p[:, kt, pair, half * H:(half + 1) * H] = \
                    G2k[kt * 128:(kt + 1) * 128, :]
    return {
        "iota16": np.ascontiguousarray(iota16),
        "positer": np.ascontiguousarray(positer),
        "featT": featT.astype(np.float32),
        "featB": np.ascontiguousarray(features_shard_T.T).astype(ml_dtypes.bfloat16),
        "w1t": w1t.astype(np.float32),
        "w2t": w2t.astype(np.float32),
        "g2p": g2p.astype(ml_dtypes.bfloat16),
        "b1t": b1t.astype(np.float32),
        "b2col": b2.reshape(K, 1).astype(np.float32),
        "onesel": np.broadcast_to(np.eye(K, dtype=np.float32)[None, :, :],
                                  (128, K, K)).astype(ml_dtypes.bfloat16),
        "ident": np.eye(128, dtype=np.float32),
        "onesrow": np.ascontiguousarray(
            np.eye(K, dtype=np.float32)[:, :, None]
            * np.ones((1, 1, 128), np.float32)),
        "scal": scal,
        "mats": mats,
    }




_CACHE = {}


def _host_reference(features, W1, b1, W2, b2, task_mats, base_threshold, beta):
    """Pure-numpy fallback (only for pathological gating densities)."""
    f = features.astype(np.float64)
    h = f @ W1.T.astype(np.float64) + b1.astype(np.float64)
    relu_h = np.maximum(h, 0.0)
    coeffs = relu_h @ W2.T.astype(np.float64) + b2.astype(np.float64)
    mask = (h > 0).astype(np.float64)
    G = W1.astype(np.float64) @ W1.T.astype(np.float64)
    u2 = np.zeros((f.shape[0], K))
    for k in range(K):
        V = mask * W2[k].astype(np.float64)[None, :]
        u2[:, k] = np.sum((V @ G) * V, axis=1)
    unc = np.sqrt(u2)
    mx = unc.max()
    if mx > 0:
        unc = unc / mx
    base = np.log1p(np.exp(np.float64(base_threshold.reshape(-1)[0])))
    th = base * (1.0 + max(float(beta.reshape(-1)[0]), 0.0) * unc)
    gated = np.where(np.abs(coeffs) < th, 0.0, coeffs)
    x = f.copy()
    for j in range(K):
        x = x + gated[:, j:j + 1] * (x @ task_mats[j].astype(np.float64))
    return x.astype(np.float32)


def _union_counts(features, W1, b1, W2, b2, base_threshold, beta):
    """Approximate per-core union-active counts (sizing check only)."""
    f = features.astype(np.float32)
    h = f @ W1.T + b1
    coeffs = np.maximum(h, 0) @ W2.T + b2
    mask = (h > 0).astype(np.float32)
    G = (W1 @ W1.T).astype(np.float32)
    u2 = np.zeros((f.shape[0], K), np.float32)
    for k in range(K):
        V = mask * W2[k][None, :]
        u2[:, k] = np.sum((V @ G) * V, axis=1)
    unc = np.sqrt(u2)
    mx = unc.max()
    if mx > 0:
        unc = unc / mx
    base = np.log1p(np.exp(np.float32(base_threshold.reshape(-1)[0])))
    th = base * (1 + max(float(beta.reshape(-1)[0]), 0.0) * unc)
    active = (np.abs(coeffs) >= th).any(axis=1)
    BT = f.shape[0] // N_CORES
    return [int(active[c * BT:(c + 1) * BT].sum()) for c in range(N_CORES)]


def kernel(features, W1, b1, W2, b2, task_mats, proj_W, base_threshold, beta,
           **_unused):
    from concourse.bass_utils import run_bass_kernel_spmd

    features = np.asarray(features, dtype=np.float32)
    W1 = np.asarray(W1, np.float32)
    b1 = np.asarray(b1, np.float32)
    W2 = np.asarray(W2, np.float32)
    b2 = np.asarray(b2, np.float32)
    task_mats = np.asarray(task_mats, np.float32)
    base_threshold = np.asarray(base_threshold, np.float32)
    beta = np.asarray(beta, np.float32)
    B = features.shape[0]
    BT = B // N_CORES
    CW = CW_DEFAULT

    # sizing check: the compact loop holds up to CW active columns per core.
    # device/host gate decisions can differ by a couple of borderline samples,
    # so require a safety margin; otherwise fall back to exact host compute.
    counts = _union_counts(features, W1, b1, W2, b2, base_threshold, beta)
    x = None
    if max(counts) <= CW - 24 and B % N_CORES == 0:
        try:
            key = ("nc", BT, CW)
            if key not in _CACHE:
                _CACHE[key] = build(BT=BT, CW=CW)
            nc = _CACHE[key]
            in_maps = []
            for c in range(N_CORES):
                shard_T = np.ascontiguousarray(features[c * BT:(c + 1) * BT].T)
                in_maps.append(make_inputs(shard_T, W1, b1, W2, b2, task_mats,
                                           base_threshold, beta, CW=CW))
            res = run_bass_kernel_spmd(nc, in_maps, core_ids=list(range(N_CORES)))
            outs = []
            for c in range(N_CORES):
                r = res.results[c]
                idx1d = r["idxout"].T.reshape(-1)[:CW]  # unwrap i = f*16 + p
                valid = idx1d >= 0
                idxs = idx1d[valid].astype(np.int64)
                xcT = r["outc"].transpose(1, 0, 2).reshape(D, CW)
                xfull = features[c * BT:(c + 1) * BT].copy()
                xfull[idxs, :] = xcT[:, valid].T
                outs.append(xfull)
            x = np.concatenate(outs, axis=0)
        except Exception as e:
            import traceback
            print(f"kernel: device path failed ({e!r}); falling back to host",
                  file=sys.stderr)
            traceback.print_exc()
            x = None
    if x is None:
        x = _host_reference(features, W1, b1, W2, b2, task_mats,
                            base_threshold, beta)

    proj_W = np.asarray(proj_W, np.float32)
    if not (proj_W.shape == (D, D) and
            np.array_equal(proj_W, np.eye(D, dtype=proj_W.dtype))):
        x = (x @ proj_W.T).astype(np.float32)
    return np.ascontiguousarray(x.astype(np.float32))


# revision 35
# speedup vs baseline: 1.3490x; 1.0010x over previous
"""AdaptiveGatingMetaNet on 8 Trainium2 NeuronCores (Bass/Tile SPMD).

Data-parallel over batch (1024 rows/core); weights replicated. Per core:
  - meta-net h in float32r (1 cyc/row PE, ~1.5e-4 rel err — gate-flip safe),
    coeffs in fp32
  - uncertainty via the Gram trick in bf16 (host-verified: zero gate flips):
    unc^2[b,k] = m_b^T diag(w2k) G diag(w2k) m_b with G = W1 W1^T
  - global max via one 512B AllReduce(max)
  - combination: ALL 8 steps run on the compacted union of active batch
    columns (CW=416, measured max 389); x is gathered once from the original
    features in a d=8-blocked layout (one 4KB-descriptor ap_gather), and the
    running x accumulates in SBUF f32 (PSUM stays free for the next rep's
    meta-net, enabling cross-rep software pipelining via wait_until tiers)
  - task matrices (16MB bf16) stream on a dedicated sync DGE ring
"""
import sys
sys.path.insert(0, "/opt/trn_rl_repo")
import numpy as np
from concourse import bass, bacc, tile, mybir

F32 = mybir.dt.float32
F32R = mybir.dt.float32r
BF16 = mybir.dt.bfloat16
I16 = mybir.dt.int16
AX = mybir.AxisListType
ALU = mybir.AluOpType
ACTF = mybir.ActivationFunctionType

D = 1024
H = 256
K = 8
KT_D = D // 128
KT_H = H // 128
N_CORES = 8
CW_DEFAULT = 416


def build(BT=1024, debug_outs=False, phase="full", repeat=1, CW=CW_DEFAULT,
          upto="full", sim=False):
    """Per-core SPMD kernel.

    phase: "full" | "null" (null = passthrough, for dispatch calibration)
    repeat: run the whole computation N times (timing amortization)
    """
    NB = BT // 128
    NS = BT // 512
    CWG = 512
    WF = CWG // 16
    WFC = CW // 16
    nc = bacc.Bacc("TRN2", target_bir_lowering=False, debug=False,
                   num_devices=N_CORES)

    featT_d = nc.dram_tensor("featT", [128, BT, KT_D], F32, kind="ExternalInput")
    featB_d = nc.dram_tensor("featB", [BT, D], BF16, kind="ExternalInput")
    w1t_d = nc.dram_tensor("w1t", [128, KT_D, H], F32, kind="ExternalInput")
    w2t_d = nc.dram_tensor("w2t", [128, KT_H, K], F32, kind="ExternalInput")
    g2p_d = nc.dram_tensor("g2p", [128, KT_H, K // 2, 2 * H], BF16,
                           kind="ExternalInput")
    b1t_d = nc.dram_tensor("b1t", [128, KT_H], F32, kind="ExternalInput")
    b2col_d = nc.dram_tensor("b2col", [K, 1], F32, kind="ExternalInput")
    onesel_d = nc.dram_tensor("onesel", [128, K, 8], BF16, kind="ExternalInput")
    ident_d = nc.dram_tensor("ident", [128, 128], F32, kind="ExternalInput")
    scal_d = nc.dram_tensor("scal", [1, 4], F32, kind="ExternalInput")
    onesrow_d = nc.dram_tensor("onesrow", [K, K, 128], F32, kind="ExternalInput")
    mats_d = nc.dram_tensor("mats", [K, 128, KT_D, D], BF16, kind="ExternalInput")
    iota16_d = nc.dram_tensor("iota16", [16, BT // 16], F32, kind="ExternalInput")
    positer_d = nc.dram_tensor("positer", [16, WF], F32, kind="ExternalInput")
    outc_d = nc.dram_tensor("outc", [128, KT_D, CW], F32, kind="ExternalOutput")
    idxout_d = nc.dram_tensor("idxout", [16, WF], F32, kind="ExternalOutput")
    if debug_outs:
        dbg_gated_d = nc.dram_tensor("dbg_gated", [K, BT], F32, kind="ExternalOutput")
        dbg_u2_d = nc.dram_tensor("dbg_u2", [K, BT], F32, kind="ExternalOutput")

    cc_in = nc.dram_tensor("cc_in", [128], F32)
    cc_out = nc.dram_tensor("cc_out", [128], F32, addr_space="Shared")

    # wait_until tiers (logical ms): phase1 of rep r at r*P1; the gating-
    # dependent tail of rep r sorts after phase1 of reps r+1 AND r+2 so the
    # in-order PE fills the collective+machinery latency with the next reps'
    # meta-net matmuls (software pipelining, depth 2).
    P1 = 0.2
    t_p1 = lambda r: r * P1
    t_mach = lambda r: r * P1 + 0.25
    t_comp = lambda r: r * P1 + 0.5

    with tile.TileContext(nc) as tc:
        with (
            tc.tile_pool(name="persist", bufs=1) as pp,
            tc.tile_pool(name="work", bufs=2) as wp,
            tc.tile_pool(name="rmats", bufs=2) as mp,
            tc.tile_pool(name="psP", bufs=1, space="PSUM") as psP,
        ):
            # ----- constants (once): gpsimd/scalar rings; the sync ring is
            # reserved for the 16MB/rep mats stream -----
            w1t = pp.tile([128, KT_D, H], F32, tag="w1t")
            nc.gpsimd.dma_start(w1t[:], w1t_d[:])
            w2t = pp.tile([128, KT_H, K], F32, tag="w2t")
            nc.gpsimd.dma_start(w2t[:], w2t_d[:])
            b1t = pp.tile([128, KT_H], F32, tag="b1t")
            nc.gpsimd.dma_start(b1t[:], b1t_d[:])
            ident = pp.tile([128, 128], F32, tag="ident")
            nc.gpsimd.dma_start(ident[:], ident_d[:])
            scal = pp.tile([1, 4], F32, tag="scal")
            nc.gpsimd.dma_start(scal[:], scal_d[:])
            b2col = pp.tile([K, 1], F32, tag="b2col")
            nc.gpsimd.dma_start(b2col[:], b2col_d[:])
            onesel = pp.tile([128, K, 8], BF16, tag="onesel")
            nc.gpsimd.dma_start(onesel[:], onesel_d[:])
            onesrow = pp.tile([K, K, 128], F32, tag="onesrow")
            nc.gpsimd.dma_start(onesrow[:], onesrow_d[:])
            iota16 = pp.tile([16, BT // 16], F32, tag="iota16")
            nc.gpsimd.dma_start(iota16[:], iota16_d[:])
            positer = pp.tile([16, WF], F32, tag="positer")
            nc.gpsimd.dma_start(positer[:], positer_d[:])
            w1tr = pp.tile([128, KT_D, H], F32R, tag="w1tr")
            nc.vector.tensor_copy(w1tr[:], w1t[:])
            # G2 pairs (diag(w2k) W1W1^T diag(w2k)) are weight-only: computed
            # on the host, streamed in as one 2MB bf16 input
            g2all = pp.tile([128, KT_H, K // 2, 2 * H], BF16, tag="g2all")
            nc.scalar.dma_start(g2all[:], g2p_d[:])

            for rep in range(repeat):
                with tc.tile_wait_until(t_p1(rep)):
                    # ----- x load in [128, b, m] layout (m fastest, so the
                    # compact gather is ONE d=8 ap_gather). Scalar ring: the
                    # sync ring is poisoned by mats buffer-wait stalls.
                    xTall = wp.tile([128, BT, KT_D], F32, tag="xTall", bufs=1,
                                    name=f"xT_{rep}")
                    nc.scalar.dma_start(xTall[:, 0:BT // 2, :],
                                        featT_d[:, 0:BT // 2, :])
                    nc.scalar.dma_start(xTall[:, BT // 2:, :],
                                        featT_d[:, BT // 2:, :])

                if phase == "null":
                    break

                # mats prefetch: 2-deep ring on the dedicated sync HWDGE ring
                matj = []
                for j in range(K):
                    t = mp.tile([128, KT_D, D], BF16, tag="matj",
                                name=f"matj{j}_{rep}")
                    nc.sync.dma_start(t[:], mats_d[j])
                    matj.append(t)

                with tc.tile_wait_until(t_p1(rep)):
                    # ----- h = relu(W1 @ x^T + b1) in f32r; maskT = (h>0) ---
                    relu = pp.tile([128, KT_H, BT], F32, tag="relu",
                                   name=f"relu_{rep}")
                    maskT = pp.tile([128, KT_H, BT], BF16, tag="maskT",
                                    name=f"maskT_{rep}")
                    for ns in range(NS):
                        nsl = slice(ns * 512, (ns + 1) * 512)
                        hacc = [psP.tile([128, 512], F32, tag=("B1", "B2")[ms],
                                         bufs=(2, 1)[ms], name=f"h{ms}_{ns}_{rep}")
                                for ms in range(KT_H)]
                        for kt in range(KT_D):
                            xr = wp.tile([128, 512], F32R, tag="xr",
                                         name=f"xr{kt}_{ns}_{rep}")
                            nc.scalar.copy(xr[:], xTall[:, nsl, kt])
                            for ms in range(KT_H):
                                nc.tensor.matmul(
                                    hacc[ms][:],
                                    w1tr[:, kt, ms * 128:(ms + 1) * 128],
                                    xr[:], start=(kt == 0), stop=(kt == KT_D - 1))
                        for ms in range(KT_H):
                            nc.scalar.activation(relu[:, ms, nsl], hacc[ms][:],
                                                 ACTF.Relu, bias=b1t[:, ms:ms + 1],
                                                 scale=1.0)
                            nc.vector.tensor_scalar(maskT[:, ms, nsl],
                                                    relu[:, ms, nsl],
                                                    0.0, None, ALU.is_gt)

                    # ----- u2T via transposed stage-1 + PE selector-ones
                    # reduce (emitted before coeffs so the collective input is
                    # ready as early as possible)
                    u2T = pp.tile([K, BT], F32, tag="u2T", name=f"u2T_{rep}")
                    for ns in range(NS):
                        nsl = slice(ns * 512, (ns + 1) * 512)
                        u2ps = psP.tile([K, 512], F32, tag="small", bufs=2,
                                        name=f"u2ps{ns}_{rep}")
                        nmm = 0
                        for k in range(K):
                            pair, half = divmod(k, 2)
                            hs = half * H
                            prods = []
                            for ms in range(KT_H):
                                xkt = psP.tile([128, 512], F32,
                                               tag=("B1", "B2")[k % 2],
                                               bufs=(2, 1)[k % 2],
                                               name=f"xkt{k}_{ms}_{ns}_{rep}")
                                for kt in range(KT_H):
                                    nc.tensor.matmul(
                                        xkt[:],
                                        g2all[:, kt, pair,
                                              hs + ms * 128:hs + (ms + 1) * 128],
                                        maskT[:, kt, nsl],
                                        start=(kt == 0), stop=(kt == KT_H - 1))
                                prod = wp.tile([128, 512], BF16, tag="prod",
                                               bufs=2,
                                               name=f"prod{k}_{ms}_{ns}_{rep}")
                                nc.vector.tensor_tensor(prod[:], xkt[:],
                                                        maskT[:, ms, nsl],
                                                        ALU.mult)
                                prods.append(prod)
                            for ms in range(KT_H):
                                nc.tensor.matmul(u2ps[:], onesel[:, k, :],
                                                 prods[ms][:],
                                                 start=(nmm == 0),
                                                 stop=(nmm == 2 * K - 1))
                                nmm += 1
                        nc.vector.tensor_copy(u2T[:, nsl], u2ps[:])

                    # ----- AllReduce max (high priority: issue the collective
                    # the moment u2T is done so peer skew starts draining)
                    with tc.high_priority():
                        lmax = wp.tile([128, 1], F32, tag="lmax",
                                       name=f"lmax_{rep}")
                        nc.vector.memset(lmax[:], 0.0)
                        nc.vector.tensor_reduce(lmax[0:K, :], u2T[:], AX.X,
                                                ALU.max)
                        nc.gpsimd.dma_start(cc_in[:], lmax[:])
                        if not sim:
                            nc.gpsimd.collective_compute(
                                "AllReduce", ALU.max,
                                replica_groups=[list(range(N_CORES))],
                                ins=[cc_in[:]], outs=[cc_out[:]])
                        gmax_col = wp.tile([1, 128], F32, tag="gmax_col",
                                           name=f"gmax_{rep}")
                        nc.gpsimd.dma_start(gmax_col[:],
                                            cc_in[:] if sim else cc_out[:])

                    # ----- coeffsT = W2 @ relu + b2 (fp32, T layout [K, BT]),
                    # only needed at gating time -----
                    coeffsT = pp.tile([K, BT], F32, tag="coeffsT",
                                      name=f"coeffsT_{rep}")
                    for ns in range(NS):
                        nsl = slice(ns * 512, (ns + 1) * 512)
                        cps = psP.tile([K, 512], F32, tag="small", bufs=2,
                                       name=f"cps{ns}_{rep}")
                        for kt in range(KT_H):
                            nc.tensor.matmul(cps[:], w2t[:, kt, :],
                                             relu[:, kt, nsl],
                                             start=(kt == 0),
                                             stop=(kt == KT_H - 1))
                        nc.scalar.activation(coeffsT[:, nsl], cps[:],
                                             ACTF.Identity, bias=b2col[:],
                                             scale=1.0)

                with tc.tile_wait_until(t_mach(rep)):
                    m2 = wp.tile([1, 1], F32, tag="m2", name=f"m2_{rep}")
                    nc.vector.tensor_reduce(m2[:], gmax_col[:], AX.X, ALU.max)
                    sq = wp.tile([1, 1], F32, tag="sqm2", name=f"sq_{rep}")
                    nc.scalar.activation(sq[:], m2[:], ACTF.Sqrt)
                    rs = wp.tile([1, 1], F32, tag="rsq", name=f"rs_{rep}")
                    nc.vector.reciprocal(rs[:], sq[:])
                    s11 = wp.tile([1, 1], F32, tag="s11", name=f"s11_{rep}")
                    nc.vector.tensor_tensor(s11[:], rs[:], scal[:, 1:2], ALU.mult)
                    sbc = wp.tile([128, 1], F32, tag="sbc", name=f"sbc_{rep}")
                    nc.gpsimd.partition_broadcast(sbc[:], s11[:])
                    basebc = wp.tile([128, 1], F32, tag="basebc",
                                     name=f"basebc_{rep}")
                    nc.gpsimd.partition_broadcast(basebc[:], scal[:, 0:1])

                    # ----- thresholds + gating (row-wise in T layout) -----
                    sqrT = wp.tile([K, BT], F32, tag="t8", name=f"sqrT_{rep}")
                    nc.scalar.activation(sqrT[:], u2T[:], ACTF.Sqrt)
                    thT = wp.tile([K, BT], F32, tag="t8", name=f"thT_{rep}")
                    nc.scalar.activation(thT[:], sqrT[:], ACTF.Identity,
                                         bias=basebc[0:K, :], scale=sbc[0:K, :])
                    absT = wp.tile([K, BT], F32, tag="t8", name=f"absT_{rep}")
                    nc.scalar.activation(absT[:], coeffsT[:], ACTF.Abs)
                    keepT = wp.tile([K, BT], F32, tag="keepT", name=f"keepT_{rep}")
                    nc.vector.tensor_tensor(keepT[:], absT[:], thT[:], ALU.is_ge)
                    gatedT16 = pp.tile([16, BT], F32, tag="gatedT16",
                                       name=f"gatedT16_{rep}")
                    nc.vector.memset(gatedT16[:], 0.0)
                    nc.vector.tensor_tensor(gatedT16[0:K, :], coeffsT[:],
                                            keepT[:], ALU.mult)
                    if debug_outs and rep == repeat - 1:
                        nc.gpsimd.dma_start(dbg_gated_d[:], gatedT16[0:K, :])
                        nc.gpsimd.dma_start(dbg_u2_d[:], u2T[:])

                    if upto == "gating":
                        continue

                    # unionK [128, NB] via PE transposes of keepT chunks
                    unionK = wp.tile([128, NB], F32, tag="unionK",
                                     name=f"unionK_{rep}")
                    for bt in range(NB):
                        tps = psP.tile([128, K], F32, tag="small", bufs=2,
                                       name=f"tpsu{bt}_{rep}")
                        nc.tensor.transpose(tps[:],
                                            keepT[:, bt * 128:(bt + 1) * 128],
                                            ident[0:K, 0:K])
                        nc.vector.tensor_reduce(unionK[:, bt:bt + 1], tps[:],
                                                AX.X, ALU.max)

                    # --- index machinery ---
                    u16 = wp.tile([16, NB, 8], F32, tag="u16", name=f"u16_{rep}")
                    for g in range(8):
                        ring = nc.scalar if g % 2 == 0 else nc.gpsimd
                        ring.dma_start(u16[:, :, g:g + 1],
                                       unionK[g * 16:(g + 1) * 16, :])
                    cand = wp.tile([16, BT // 16], F32, tag="cand",
                                   name=f"cand_{rep}")
                    # u16 layout: [p16, g, bt] maps to b = bt*128 + g*16 + p16;
                    # iota wrapped layout needs b = f*16 + p16, f = bt*8 + g.
                    u16flat = u16[:].rearrange("p b g -> p (b g)")
                    nc.vector.tensor_tensor(cand[:], iota16[:], u16flat, ALU.mult)
                    um1 = wp.tile([16, BT // 16], F32, tag="um1", name=f"um1_{rep}")
                    nc.vector.tensor_scalar(um1[:], u16flat, 1.0, None,
                                            ALU.subtract)
                    nc.vector.tensor_tensor(cand[:], cand[:], um1[:], ALU.add)
                    idxf = wp.tile([16, WF], F32, tag="idxf", name=f"idxf_{rep}")
                    cnt = wp.tile([1, 1], mybir.dt.uint32, tag="cnt",
                                  name=f"cnt_{rep}")
                    nc.gpsimd.sparse_gather(idxf[:], cand[:], num_found=cnt[:])
                    # mask junk tail: idx = idx if pos < cnt else -1
                    cntf = wp.tile([1, 1], F32, tag="cntf", name=f"cntf_{rep}")
                    nc.vector.tensor_copy(cntf[:], cnt[:])
                    cntbc = wp.tile([16, 1], F32, tag="cntbc", name=f"cntbc_{rep}")
                    nc.gpsimd.partition_broadcast(cntbc[:], cntf[:])
                    posok = wp.tile([16, WF], F32, tag="posok", name=f"posok_{rep}")
                    nc.vector.tensor_scalar(posok[:], positer[:], cntbc[:], None,
                                            ALU.is_lt)
                    nc.vector.tensor_tensor(idxf[:], idxf[:], posok[:], ALU.mult)
                    pm1 = wp.tile([16, WF], F32, tag="pm1", name=f"pm1_{rep}")
                    nc.vector.tensor_scalar(pm1[:], posok[:], 1.0, None,
                                            ALU.subtract)
                    nc.vector.tensor_tensor(idxf[:], idxf[:], pm1[:], ALU.add)
                    nc.gpsimd.dma_start(idxout_d[:], idxf[:])
                    idxf0 = wp.tile([16, WF], F32, tag="idxf0", name=f"idxf0_{rep}")
                    nc.vector.tensor_scalar(idxf0[:], idxf[:], 0.0, None, ALU.max)
                    idx16 = wp.tile([16, WF], I16, tag="idx16", name=f"idx16_{rep}")
                    nc.vector.tensor_copy(idx16[:], idxf0[:])
                    idxrep = wp.tile([128, WF], I16, tag="idxrep",
                                     name=f"idxrep_{rep}")
                    for g in range(8):
                        ring = nc.scalar if g % 2 == 0 else nc.gpsimd
                        ring.dma_start(idxrep[g * 16:(g + 1) * 16, :], idx16[:])

                    # gatedTc: compact gated rows; broadcast each row to all
                    # 128 partitions in bf16 (stage at partition 0 first —
                    # this concourse requires partition-0 sources)
                    gatedTc = wp.tile([16, CW], F32, tag="gatedTc", bufs=1,
                                      name=f"gatedTc_{rep}")
                    nc.gpsimd.ap_gather(gatedTc[:], gatedT16[:], idx16[:, 0:WFC],
                                        channels=16, num_elems=BT, d=1,
                                        num_idxs=CW)
                    gball = wp.tile([128, K, CW], BF16, tag="gball", bufs=1,
                                    name=f"gball_{rep}")
                    for j in range(K):
                        gbps = psP.tile([128, CW], F32, tag="small", bufs=2,
                                        name=f"gbps{j}_{rep}")
                        nc.tensor.matmul(gbps[:], onesrow[:, j, :],
                                         gatedTc[0:K, :], start=True, stop=True)
                        nc.vector.tensor_copy(gball[:, j, :], gbps[:])

                    # gather compacted ORIGINAL x straight from DRAM:
                    # one transposing dma_gather of CWG 2KB rows (bf16)
                    xgb = wp.tile([128, KT_D, CWG], BF16, tag="xgb", bufs=2,
                                  name=f"xgb_{rep}")
                    nc.gpsimd.dma_gather(xgb[:], featB_d[:], idxrep[:],
                                         CWG, CWG, D, transpose=True)

                # ----- compact combination loop, ALL 8 steps; running x in
                # SBUF f32 (xc), updates accumulate per step via DVE/gpsimd
                # adds; PSUM holds only the transient per-(j,m) matmul result.
                with tc.tile_wait_until(t_comp(rep)):
                    xc = wp.tile([128, KT_D, CW], F32, tag="xc", bufs=1,
                                 name=f"xc_{rep}")

                    def rc_tile(i, nm):
                        return wp.tile([128, KT_D, CW], BF16,
                                       tag=("rcA", "rcB")[i % 2], bufs=1,
                                       name=nm)
                    rc = rc_tile(0, f"rc0_{rep}")
                    for m in range(KT_D):
                        nc.vector.tensor_tensor(
                            rc[:, m, :], xgb[:, m, 0:CW], gball[:, 0, :],
                            ALU.mult)
                    for j in range(K):
                        rcn = rc_tile(j + 1, f"rc{j+1}_{rep}") if j + 1 < K \
                            else None
                        for m in range(KT_D):
                            ps = psP.tile([128, CW], F32, tag="A", bufs=3,
                                          name=f"cps{j}_{m}_{rep}")
                            for kt in range(KT_D):
                                nc.tensor.matmul(
                                    ps[:], matj[j][:, kt, m * 128:(m + 1) * 128],
                                    rc[:, kt, :],
                                    start=(kt == 0), stop=(kt == KT_D - 1))
                            if j == 0:
                                nc.vector.tensor_tensor(
                                    xc[:, m, :], xgb[:, m, 0:CW], ps[:], ALU.add)
                            else:
                                # PSUM reads must stay off gpsimd
                                nc.vector.tensor_tensor(
                                    xc[:, m, :], xc[:, m, :], ps[:], ALU.add)
                            if rcn is not None:
                                nc.vector.tensor_tensor(
                                    rcn[:, m, :], xc[:, m, :],
                                    gball[:, j + 1, :], ALU.mult)
                        rc = rcn

                    if phase != "null":
                        nc.gpsimd.dma_start(outc_d[:], xc[:])

    nc.finalize()
    return nc


def make_inputs(features_shard_T, W1, b1, W2, b2, task_mats, base_threshold, beta,
                CW=CW_DEFAULT):
    import ml_dtypes
    BT = features_shard_T.shape[1]
    featT = np.ascontiguousarray(
        features_shard_T.reshape(KT_D, 128, BT).transpose(1, 2, 0))
    w1t = np.ascontiguousarray(W1.T.reshape(KT_D, 128, H).transpose(1, 0, 2))
    w2t = np.ascontiguousarray(W2.T.reshape(KT_H, 128, K).transpose(1, 0, 2))
    b1t = np.ascontiguousarray(b1.reshape(KT_H, 128).T)
    base_sp = np.log1p(np.exp(np.float32(base_threshold[0]))).astype(np.float32)
    rbeta = np.maximum(np.float32(beta[0]), 0).astype(np.float32)
    scal = np.array([[base_sp, base_sp * rbeta, 0, 0]], np.float32)
    mats = np.ascontiguousarray(
        task_mats.reshape(K, KT_D, 128, D).transpose(0, 2, 1, 3)).astype(ml_dtypes.bfloat16)
    iota16 = (np.arange(BT).reshape(BT // 16, 16).T).astype(np.float32)
    positer = (np.arange(512).reshape(32, 16).T).astype(np.float32)
    # host-side weight-only precompute: G2 pairs in the device layout
    # g2p[p, kt, pair, half*H + h'] = W2[k,h] * G[h,h'] * W2[k,h']
    # with h = kt*128 + p, k = 2*pair + half
    G = (W1.astype(np.float32) @ W1.T.astype(np.float32))
    g2p = np.empty((128, KT_H, K // 2, 2 * H), np.float32)
    for pair in range(K // 2):
        for half in range(2):
            k = 2 * pair + half
            G2k = W2[k][:, None] * G * W2[k][None, :]
            for kt in range(KT_H):
                g2p[:, kt, pair, half * H:(half + 1) * H] = \
                    G2k[kt * 128:(kt + 1) * 128, :]
    return {
        "iota16": np.ascontiguousarray(iota16),
        "positer": np.ascontiguousarray(positer),
        "featT": featT.astype(np.float32),
        "featB": np.ascontiguousarray(features_shard_T.T).astype(ml_dtypes.bfloat16),
        "w1t": w1t.astype(np.float32),
        "w2t": w2t.astype(np.float32),
        "g2p": g2p.astype(ml_dtypes.bfloat16),
        "b1t": b1t.astype(np.float32),
        "b2col": b2.reshape(K, 1).astype(np.float32),
        "onesel": np.broadcast_to(np.eye(K, dtype=np.float32)[None, :, :],
                                  (128, K, K)).astype(ml_dtypes.bfloat16),
        "ident": np.eye(128, dtype=np.float32),
        "onesrow": np.ascontiguousarray(
            np.eye(K, dtype=np.float32)[:, :, None]
            * np.ones((1, 1, 128), np.float32)),
        "scal": scal,
        "mats": mats,
    }




_CACHE = {}


def _host_reference(features, W1, b1, W2, b2, task_mats, base_threshold, beta):
    """Pure-numpy fallback (only for pathological gating densities)."""
    f = features.astype(np.float64)
    h = f @ W1.T.astype(np.float64) + b1.astype(np.float64)
    relu_h = np.maximum(h, 0.0)
    coeffs = relu_h @ W2.T.astype(np.float64) + b2.astype(np.float64)
    mask = (h > 0).astype(np.float64)
    G = W1.astype(np.float64) @ W1.T.astype(np.float64)
    u2 = np.zeros((f.shape[0], K))
    for k in range(K):
        V = mask * W2[k].astype(np.float64)[None, :]
        u2[:, k] = np.sum((V @ G) * V, axis=1)
    unc = np.sqrt(u2)
    mx = unc.max()
    if mx > 0:
        unc = unc / mx
    base = np.log1p(np.exp(np.float64(base_threshold.reshape(-1)[0])))
    th = base * (1.0 + max(float(beta.reshape(-1)[0]), 0.0) * unc)
    gated = np.where(np.abs(coeffs) < th, 0.0, coeffs)
    x = f.copy()
    for j in range(K):
        x = x + gated[:, j:j + 1] * (x @ task_mats[j].astype(np.float64))
    return x.astype(np.float32)


def _union_counts(features, W1, b1, W2, b2, base_threshold, beta):
    """Approximate per-core union-active counts (sizing check only)."""
    f = features.astype(np.float32)
    h = f @ W1.T + b1
    coeffs = np.maximum(h, 0) @ W2.T + b2
    mask = (h > 0).astype(np.float32)
    G = (W1 @ W1.T).astype(np.float32)
    u2 = np.zeros((f.shape[0], K), np.float32)
    for k in range(K):
        V = mask * W2[k][None, :]
        u2[:, k] = np.sum((V @ G) * V, axis=1)
    unc = np.sqrt(u2)
    mx = unc.max()
    if mx > 0:
        unc = unc / mx
    base = np.log1p(np.exp(np.float32(base_threshold.reshape(-1)[0])))
    th = base * (1 + max(float(beta.reshape(-1)[0]), 0.0) * unc)
    active = (np.abs(coeffs) >= th).any(axis=1)
    BT = f.shape[0] // N_CORES
    return [int(active[c * BT:(c + 1) * BT].sum()) for c in range(N_CORES)]


def kernel(features, W1, b1, W2, b2, task_mats, proj_W, base_threshold, beta,
           **_unused):
    from concourse.bass_utils import run_bass_kernel_spmd

    features = np.asarray(features, dtype=np.float32)
    W1 = np.asarray(W1, np.float32)
    b1 = np.asarray(b1, np.float32)
    W2 = np.asarray(W2, np.float32)
    b2 = np.asarray(b2, np.float32)
    task_mats = np.asarray(task_mats, np.float32)
    base_threshold = np.asarray(base_threshold, np.float32)
    beta = np.asarray(beta, np.float32)
    B = features.shape[0]
    BT = B // N_CORES
    CW = CW_DEFAULT

    # sizing check: the compact loop holds up to CW active columns per core.
    # device/host gate decisions can differ by a couple of borderline samples,
    # so require a safety margin; otherwise fall back to exact host compute.
    counts = _union_counts(features, W1, b1, W2, b2, base_threshold, beta)
    x = None
    if max(counts) <= CW - 24 and B % N_CORES == 0:
        try:
            key = ("nc", BT, CW)
            if key not in _CACHE:
                _CACHE[key] = build(BT=BT, CW=CW)
            nc = _CACHE[key]
            in_maps = []
            for c in range(N_CORES):
                shard_T = np.ascontiguousarray(features[c * BT:(c + 1) * BT].T)
                in_maps.append(make_inputs(shard_T, W1, b1, W2, b2, task_mats,
                                           base_threshold, beta, CW=CW))
            res = run_bass_kernel_spmd(nc, in_maps, core_ids=list(range(N_CORES)))
            outs = []
            for c in range(N_CORES):
                r = res.results[c]
                idx1d = r["idxout"].T.reshape(-1)[:CW]  # unwrap i = f*16 + p
                valid = idx1d >= 0
                idxs = idx1d[valid].astype(np.int64)
                xcT = r["outc"].transpose(1, 0, 2).reshape(D, CW)
                xfull = features[c * BT:(c + 1) * BT].copy()
                xfull[idxs, :] = xcT[:, valid].T
                outs.append(xfull)
            x = np.concatenate(outs, axis=0)
        except Exception as e:
            import traceback
            print(f"kernel: device path failed ({e!r}); falling back to host",
                  file=sys.stderr)
            traceback.print_exc()
            x = None
    if x is None:
        x = _host_reference(features, W1, b1, W2, b2, task_mats,
                            base_threshold, beta)

    proj_W = np.asarray(proj_W, np.float32)
    if not (proj_W.shape == (D, D) and
            np.array_equal(proj_W, np.eye(D, dtype=proj_W.dtype))):
        x = (x @ proj_W.T).astype(np.float32)
    return np.ascontiguousarray(x.astype(np.float32))
